# revision 1
# baseline (speedup 1.0000x reference)
"""Expert-parallel MoE layer for Trainium2 (Bass/Tile, 8 NeuronCores).

Strategy (sharding hardcoded for B=4, T=2048, C=1024, E=8, H=2728, top_k=2):
  - Expert-parallel: core e owns expert e's weights (w1/w2/w3[e]).
  - Host computes the router *selection* only (top-2 expert ids per token;
    verified identical across numpy/jax-cpu/jax-neuron fp32 paths for this
    regime) and performs the all-to-all token dispatch/combine as the
    shard/unshard step.
  - Each core, on device: recomputes gate logits for its tokens (gate_w is
    replicated), derives its softmax combine weight g = exp(l_e - m1) /
    (exp(m1 - m1) + exp(m2 - m1)) per token, computes the full expert FFN
    y = (silu(x@w1.T) * (x@w3.T)) @ w2.T, scales by g, and returns the
    per-expert partial outputs, which the host scatter-adds.

Matmuls run in float32r (fp32 storage, full PE rate, ~11-bit mantissa);
inputs are pre-rounded on the host so DRAM tensors can be declared float32r
and DMA'd straight into PE-ready SBUF tiles. Layouts are pre-arranged on the
host (partition-major) so big transfers are single contiguous DMAs, spread
across several engine queues.
"""

import os
import sys
from contextlib import ExitStack

import numpy as np

for _p in ("/opt/trn_rl_repo", "/root/.axon_site/_ro/trn_rl_repo"):
    if os.path.isdir(_p) and _p not in sys.path:
        sys.path.insert(0, _p)

import concourse.mybir as mybir
import concourse.tile as tile
from concourse.tile_rust import add_dep_helper
from concourse import bacc
from concourse.bass_utils import run_bass_kernel_spmd

FP32 = mybir.dt.float32
F32R = mybir.dt.float32r  # fp32 storage, PE matmul at full rate (~11-bit mantissa)
ALU = mybir.AluOpType
AF = mybir.ActivationFunctionType
AX = mybir.AxisListType

E = 8            # experts == cores
C = 1024         # model dim
H0 = 2728        # ffn hidden dim
KC = C // 128    # 8 contraction tiles over C
KH = (H0 + 127) // 128  # 22 tiles over padded H
HP = KH * 128    # 2816
TT = 512         # max token tile (fp32 PSUM bank = 512 floats)
# s reload split: 4 groups of h-tiles so phase B's first matmuls only wait
# on a quarter of each token tile's s block
S_GRP = [(0, 2), (2, 6), (8, 7), (15, 7)]
CAP_MAX = 2304   # per-launch token cap (SBUF budget); split into runs beyond

_CACHE = {}
LAST_RESULTS = None


def _token_tiles(cap):
    # all tiles >= 256 wide: float32r matmuls below 256 columns run at a
    # 4x/2x cycles-per-row penalty, so a narrow tail tile costs as much as
    # a full 512 tile. Sub-512 tiles go FIRST: the first matmul's DMA
    # dependency is smaller, so the PE starts (and ramps) earlier.
    widths = []
    left = cap
    while left > 640:
        widths.append(TT)
        left -= TT
    if left == 640:
        widths = [384, 256] + widths
    elif left > 0:
        widths = [left] + widths
    widths.sort()
    tiles = []
    off = 0
    for w in widths:
        tiles.append((off, w))
        off += w
    return tiles


def _preround(a):
    """Round fp32 array to float32r precision (round-to-nearest with the 12
    low mantissa bits dropped) so DRAM tensors can be declared float32r and
    DMA'd straight into PE-ready SBUF tiles with no on-device conversion."""
    v = np.ascontiguousarray(a, dtype=np.float32).view(np.uint32)
    r = ((v.astype(np.uint64) + 0x800) & 0xFFFFF000).astype(np.uint32)
    return r.view(np.float32)


def _build(cap):
    """Build + compile the SPMD program for `cap` tokens per core."""
    assert cap % 128 == 0
    NTT = cap // 128        # 128-token tiles (gate)
    tiles = _token_tiles(cap)
    nc = bacc.Bacc("TRN2", target_bir_lowering=False, debug=False, num_devices=E)

    xs = nc.dram_tensor("xs", [KC, 128, cap], F32R, kind="ExternalInput").ap()
    w1s = nc.dram_tensor("w1s", [KH, 128, C], F32R, kind="ExternalInput").ap()
    w3s = nc.dram_tensor("w3s", [KH, 128, C], F32R, kind="ExternalInput").ap()
    w2s = nc.dram_tensor("w2s", [KC, 128, KH * 128], F32R, kind="ExternalInput").ap()
    gws = nc.dram_tensor("gws", [128, KC, E], F32R, kind="ExternalInput").ap()
    esl = nc.dram_tensor("esl", [128, E], FP32, kind="ExternalInput").ap()
    yt = nc.dram_tensor("yt", [KC, 128, cap], FP32, kind="ExternalOutput").ap()

    with tile.TileContext(nc) as tc, ExitStack() as top:
        dramp = top.enter_context(tc.tile_pool(name="dram", bufs=1, space="DRAM"))
        constp = top.enter_context(tc.tile_pool(name="const", bufs=1))
        gresp = top.enter_context(tc.tile_pool(name="gres", bufs=1))

        s_dram = dramp.tile([128, KH, cap], F32R)
        g_dram = dramp.tile([cap], FP32)

        # gate consts ride the ACT queue: off the sync head (which must
        # deliver w[h0]+x[t0] ASAP) but still landed long before the gate
        gw_sb = constp.tile([128, KC, E], F32R)
        nc.scalar.dma_start(gw_sb[:], gws[:])
        es_sb = constp.tile([128, E], FP32)
        nc.scalar.dma_start(es_sb[:], esl[:])

        gcol = gresp.tile([128, NTT], FP32)

        # w2 is resident for the whole kernel; its loads are interleaved
        # into phase A's h-loop (below) so they hide behind compute without
        # starving the startup x/w1/w3 streams of DMA bandwidth
        w2p = top.enter_context(tc.tile_pool(name="w2res", bufs=1))
        w2_sb = [w2p.tile([128, KH, 128], F32R, tag=f"w2_{c}",
                          name=f"w2_sb_{c}") for c in range(KC)]

        anchor = None
        with ExitStack() as pha:
            xp = pha.enter_context(tc.tile_pool(name="xres", bufs=1))
            x_sb = [xp.tile([128, cap], F32R, tag=f"x{k}", name=f"x_sb{k}")
                    for k in range(KC)]
            # startup-critical loads share the SYNC queue in exact
            # consumption order (per-queue DMA processing is FIFO). They
            # must NOT ride the ACT queue: ACT's in-order sequencer would
            # sit in dma-issue instructions instead of running phase A's
            # silu ops, stalling PSUM slot recycling and starving the PE.
            wst = pha.enter_context(tc.tile_pool(name="wst", bufs=4))
            w_pre = {}
            for h in range(min(4, KH)):
                w1p_ = wst.tile([128, C], F32R, tag="w1", name=f"w1_sb{h}")
                w3p_ = wst.tile([128, C], F32R, tag="w3", name=f"w3_sb{h}")
                w_pre[h] = (w1p_, w3p_)
            nc.sync.dma_start(w_pre[0][0][:], w1s[0])
            nc.sync.dma_start(w_pre[0][1][:], w3s[0])
            w_loaded = {0}
            for ti, (to, tw) in enumerate(tiles):
                for k in range(KC):
                    nc.sync.dma_start(x_sb[k][:, to:to + tw],
                                      xs[k][:, to:to + tw])
                # weight rows for h=1..3 slot into the x stream in
                # consumption order (per-queue processing is FIFO, and the
                # tiles are pre-allocated so nothing delays the issue)
                hh = ti + 1
                if ti <= 2 and hh in w_pre and hh not in w_loaded:
                    nc.sync.dma_start(w_pre[hh][0][:], w1s[hh])
                    nc.sync.dma_start(w_pre[hh][1][:], w3s[hh])
                    w_loaded.add(hh)
            for hh in sorted(set(w_pre) - w_loaded - {0}):
                nc.sync.dma_start(w_pre[hh][0][:], w1s[hh])
                nc.sync.dma_start(w_pre[hh][1][:], w3s[hh])

            # ---- gate: logits -> per-token combine weight for this expert.
            # Emitted (below) after the startup h-batch: its ACT exp ops for
            # the last token tiles would otherwise precede phase A's silus
            # in ACT's in-order stream while waiting on the last x chunks.
            psg = pha.enter_context(tc.tile_pool(name="psg", bufs=2, space="PSUM"))
            gtmp = pha.enter_context(tc.tile_pool(name="gtmp", bufs=2))

            def emit_gate():
              for tt in range(NTT):
                  sl = slice(tt * 128, (tt + 1) * 128)
                  pl = psg.tile([128, E], FP32, tag="pl", name=f"pl{tt}")
                  for k in range(KC):
                      nc.tensor.matmul(pl[:], x_sb[k][:, sl], gw_sb[:, k, :],
                                       start=(k == 0), stop=(k == KC - 1))
                  l_sb = gtmp.tile([128, E], FP32, tag="l", name=f"l{tt}")
                  nc.vector.tensor_copy(l_sb[:], pl[:])
                  m1 = gtmp.tile([128, 1], FP32, tag="m1", name=f"m1_{tt}")
                  nc.vector.reduce_max(m1[:], l_sb[:], axis=AX.X)
                  eq = gtmp.tile([128, E], FP32, tag="eq", name=f"eq{tt}")
                  nc.vector.tensor_scalar(eq[:], l_sb[:], m1[:], None, ALU.is_equal)
                  eqb = gtmp.tile([128, E], FP32, tag="eqb", name=f"eqb{tt}")
                  nc.vector.tensor_scalar(eqb[:], eq[:], 1e30, None, ALU.mult)
                  msk = gtmp.tile([128, E], FP32, tag="msk", name=f"msk{tt}")
                  nc.vector.tensor_tensor(msk[:], l_sb[:], eqb[:], op=ALU.subtract)
                  m2 = gtmp.tile([128, 1], FP32, tag="m2", name=f"m2_{tt}")
                  nc.vector.reduce_max(m2[:], msk[:], axis=AX.X)
                  sel = gtmp.tile([128, E], FP32, tag="sel", name=f"sel{tt}")
                  nc.vector.tensor_tensor(sel[:], l_sb[:], es_sb[:], op=ALU.mult)
                  le = gtmp.tile([128, 1], FP32, tag="le", name=f"le{tt}")
                  nc.vector.reduce_sum(le[:], sel[:], axis=AX.X)
                  nm1 = gtmp.tile([128, 1], FP32, tag="nm1", name=f"nm1_{tt}")
                  nc.vector.tensor_scalar(nm1[:], m1[:], -1.0, None, ALU.mult)
                  ea = gtmp.tile([128, 1], FP32, tag="ea", name=f"ea{tt}")
                  nc.scalar.activation(ea[:], le[:], AF.Exp, bias=nm1[:])
                  eb = gtmp.tile([128, 1], FP32, tag="eb", name=f"eb{tt}")
                  nc.scalar.activation(eb[:], m2[:], AF.Exp, bias=nm1[:])
                  den = gtmp.tile([128, 1], FP32, tag="den", name=f"den{tt}")
                  nc.vector.tensor_scalar(den[:], eb[:], 1.0, None, ALU.add)
                  rec = gtmp.tile([128, 1], FP32, tag="rec", name=f"rec{tt}")
                  nc.vector.reciprocal(rec[:], den[:])
                  nc.vector.tensor_tensor(gcol[:, tt:tt + 1], ea[:], rec[:],
                                          op=ALU.mult)
              # g: [128-token partition] columns -> flat HBM (gpsimd queue:
              # only ready once the gate finishes; would head-block an
              # in-order HWDGE queue shared with the weight streams). The
              # row reload + partition broadcast happen at the top of B.
              nc.gpsimd.dma_start(
                  g_dram.rearrange("(t p) -> p t", p=128), gcol[:])

            # ---- phase A: s = silu(x@w1.T) * (x@w3.T), token-major in HBM.
            # Emission order: h0..h3 over all tiles but the last, then the
            # last tile for h0..h3, then h4+. Pool slots recycle in
            # allocation order, so putting the late-arriving last x tile's
            # work in a trailing batch keeps its stall out of the slot
            # chain that feeds h2/h3 during the x stream.
            psa = pha.enter_context(tc.tile_pool(name="psA", bufs=3, space="PSUM"))
            stg = pha.enter_context(tc.tile_pool(name="stg", bufs=3))
            anchors = {}

            def emit_ht(h, t, w1_sb, w3_sb):
                nonlocal anchor
                to, tw = tiles[t]
                p1 = psa.tile([128, TT], FP32, tag="p1", name=f"p1_{h}_{t}")
                p3 = psa.tile([128, TT], FP32, tag="p3", name=f"p3_{h}_{t}")
                for k in range(KC):
                    nc.tensor.matmul(p1[:, :tw],
                                     w1_sb[:, k * 128:(k + 1) * 128],
                                     x_sb[k][:, to:to + tw],
                                     start=(k == 0), stop=(k == KC - 1))
                for k in range(KC):
                    anchor = nc.tensor.matmul(
                        p3[:, :tw], w3_sb[:, k * 128:(k + 1) * 128],
                        x_sb[k][:, to:to + tw],
                        start=(k == 0), stop=(k == KC - 1))
                anchors[(h, t)] = anchor
                sa = stg.tile([128, TT], FP32, tag="sa", name=f"sa{h}_{t}")
                nc.scalar.activation(sa[:, :tw], p1[:, :tw], AF.Silu)
                so = stg.tile([128, TT], F32R, tag="so", name=f"so{h}_{t}")
                nc.vector.tensor_tensor(so[:, :tw], sa[:, :tw], p3[:, :tw],
                                        op=ALU.mult)
                nc.scalar.dma_start(s_dram[:, h, to:to + tw], so[:, :tw])

            last = len(tiles) - 1
            npre = min(4, KH)
            seq = [(h, t) for h in range(npre) for t in range(last)]
            seq += [(h, last) for h in range(npre)]
            w_cur = dict(w_pre)
            for h, t in seq:
                emit_ht(h, t, *w_cur[h])
            emit_gate()
            seq = [(h, t) for h in range(npre, KH) for t in range(len(tiles))]
            for h, t in seq:
                if h not in w_cur:
                    w1_sb = wst.tile([128, C], F32R, tag="w1", name=f"w1_sb{h}")
                    nc.sync.dma_start(w1_sb[:], w1s[h])
                    w3_sb = wst.tile([128, C], F32R, tag="w3", name=f"w3_sb{h}")
                    nc.sync.dma_start(w3_sb[:], w3s[h])
                    w_cur[h] = (w1_sb, w3_sb)
                    if npre <= h <= npre + 2 * KC - 2 and (h - npre) % 2 == 0:
                        c = (h - npre) // 2
                        w2dma = nc.gpsimd.dma_start(
                            w2_sb[c][:],
                            w2s[c].rearrange("p (h j) -> p h j", h=KH))
                        # hold each w2 load back until phase A is under way,
                        # so the startup x/w1/w3 streams keep the DMA
                        # engines to themselves
                        add_dep_helper(w2dma.ins, anchor.ins,
                                       reason="delay w2 prefetch")
                emit_ht(h, t, *w_cur[h])

        # ---- phase B: y = (s @ w2.T) * g ----
        with ExitStack() as phb:
            gbc = phb.enter_context(tc.tile_pool(name="gbc", bufs=1))
            g_sb = []
            for t, (to, tw) in enumerate(tiles):
                grow = gbc.tile([1, TT], FP32, tag="grow", name=f"grow{t}",
                                bufs=2)
                nc.gpsimd.dma_start(grow[0:1, :tw], g_dram[to:to + tw])
                gt = gbc.tile([128, tw], FP32, tag=f"g{t}", name=f"g_sb{t}")
                nc.gpsimd.partition_broadcast(gt[:], grow[0:1, :tw])
                g_sb.append(gt)
            sinp = phb.enter_context(tc.tile_pool(name="sin", bufs=2))
            psb = phb.enter_context(tc.tile_pool(name="psB", bufs=3, space="PSUM"))
            yp = phb.enter_context(tc.tile_pool(name="yst", bufs=4))
            # smallest tile first (quick phase entry) and second-smallest
            # last (short final drain); the middle in arbitrary order
            b_order = sorted(range(len(tiles)),
                             key=lambda i: (i != 0, -tiles[i][1]))
            for t in b_order:
                to, tw = tiles[t]
                s_t = []
                for q, (h0, hn) in enumerate(S_GRP):
                    sq = sinp.tile([128, hn, TT], F32R, tag=f"sq{q}",
                                   name=f"s_q{q}_{t}")
                    nc.scalar.dma_start(sq[:, :, :tw],
                                        s_dram[:, h0:h0 + hn, to:to + tw])
                    s_t.append(sq)
                for c in range(KC):
                    py = psb.tile([128, TT], FP32, tag="py", name=f"py{t}_{c}")
                    for q, (h0, hn) in enumerate(S_GRP):
                        for hh in range(hn):
                            h = h0 + hh
                            nc.tensor.matmul(py[:, :tw], w2_sb[c][:, h, :],
                                             s_t[q][:, hh, :tw],
                                             start=(h == 0),
                                             stop=(h == KH - 1))
                    yb = yp.tile([128, TT], FP32, tag="y", name=f"yb{t}_{c}")
                    nc.vector.tensor_tensor(yb[:, :tw], py[:, :tw], g_sb[t][:],
                                            op=ALU.mult)
                    nc.sync.dma_start(yt[c, :, to:to + tw], yb[:, :tw])

    nc.compile()
    return nc


def kernel(x, gate_w, w1, w2, w3, top_k):
    global LAST_RESULTS
    x = np.asarray(x, dtype=np.float32)
    gw = np.asarray(gate_w, dtype=np.float32)
    w1 = np.asarray(w1, dtype=np.float32)
    w2 = np.asarray(w2, dtype=np.float32)
    w3 = np.asarray(w3, dtype=np.float32)
    assert int(np.asarray(top_k)) == 2
    Bb, T, Cc = x.shape
    N = Bb * T
    assert Cc == C and w1.shape == (E, H0, C)

    xf = np.ascontiguousarray(x.reshape(N, C))
    # Router selection on host (dispatch is the sharding step); the gate
    # weights actually applied to the output are recomputed on device.
    logits = xf @ gw.T
    order = np.argsort(-logits, axis=1, kind="stable")[:, :2]
    tok = [np.nonzero((order == e).any(axis=1))[0] for e in range(E)]

    gws_np = _preround(
        np.ascontiguousarray(gw.T.reshape(KC, 128, E).transpose(1, 0, 2)))
    wmaps = []
    for e in range(E):
        w1t = np.zeros((C, HP), np.float32)
        w1t[:, :H0] = w1[e].T
        w1s_np = _preround(np.ascontiguousarray(
            w1t.reshape(KC, 128, KH, 128).transpose(2, 1, 0, 3)).reshape(KH, 128, C))
        w3t = np.zeros((C, HP), np.float32)
        w3t[:, :H0] = w3[e].T
        w3s_np = _preround(np.ascontiguousarray(
            w3t.reshape(KC, 128, KH, 128).transpose(2, 1, 0, 3)).reshape(KH, 128, C))
        w2t = np.zeros((HP, C), np.float32)
        w2t[:H0] = w2[e].T
        w2s_np = _preround(np.ascontiguousarray(
            w2t.reshape(KH, 128, KC, 128).transpose(2, 1, 0, 3)).reshape(KC, 128, KH * 128))
        es_np = np.zeros((128, E), np.float32)
        es_np[:, e] = 1.0
        wmaps.append({"w1s": w1s_np, "w3s": w3s_np, "w2s": w2s_np,
                      "gws": gws_np, "esl": es_np})

    out = np.zeros((N, C), np.float32)
    # normally one launch; if an expert ever holds > CAP_MAX tokens, split
    # tokens into several SPMD launches (FFN + gate weight are per-token)
    nchunk = (max(t.size for t in tok) + CAP_MAX - 1) // CAP_MAX
    for ci in range(nchunk):
        tokc = [t[(ci * t.size) // nchunk:((ci + 1) * t.size) // nchunk]
                for t in tok]
        cap = max(TT, ((max(t.size for t in tokc) + 127) // 128) * 128)
        if cap not in _CACHE:
            _CACHE[cap] = _build(cap)
        nc = _CACHE[cap]
        in_maps = []
        for e in range(E):
            idx = tokc[e]
            n = idx.size
            xe = np.zeros((cap, C), np.float32)
            xe[:n] = xf[idx]
            xs_np = _preround(np.ascontiguousarray(xe.T).reshape(KC, 128, cap))
            in_maps.append({"xs": xs_np, **wmaps[e]})

        trace = os.environ.get("BASS_MOE_TRACE", "0") == "1"
        try:
            res = run_bass_kernel_spmd(nc, in_maps, core_ids=list(range(E)),
                                       trace=trace)
        except ModuleNotFoundError:
            # NTFF profile hook unavailable here; run untraced.
            res = run_bass_kernel_spmd(nc, in_maps, core_ids=list(range(E)))
        LAST_RESULTS = res

        for e in range(E):
            idx = tokc[e]
            n = idx.size
            ye = res.results[e]["yt"].reshape(C, cap).T
            out[idx] += ye[:n]
    return out.reshape(Bb, T, C)



# revision 2
# speedup vs baseline: 1.2964x; 1.2964x over previous
"""Expert-parallel MoE layer for Trainium2 (Bass/Tile, 8 NeuronCores).

Strategy (hardcoded for B=4, T=2048, C=1024, E=8, H=2728, top_k=2):
  - Expert-parallel: core e owns expert e's weights (w1/w2/w3[e]).
  - Host computes the router (top-2 ids AND the softmax combine weights in
    exact fp32) and performs the all-to-all token dispatch/combine as the
    shard/unshard step. The per-token gate weight rides in as a small fp32
    vector, pre-scaled by the fp8 scale factors.
  - Each core computes the full expert FFN y = (silu(x@w1.T) * (x@w3.T))
    @ w2.T * g for its tokens, entirely in fp8-e4m3 DoubleRow matmuls
    (K=256 per instruction, 0.5 cycles/column — 2x the fp32r MAC rate).

Precision: every GEMM uses a 3-term hi/lo split, all at one shared scale so
the three products accumulate in a single PSUM chain:
    A@W ~= Ah@Wh + Al@Wh + Ah@Wl      (drops only the Al@Wl term, ~7e-4)
with Ah = e4m3(A*S), Al = e4m3(A*S - Ah). End-to-end rel err ~2e-3 vs the
2e-2 gate. x/w splits happen on host; the phase-A output s is split on
device (ACT copy for the hi part, DVE subtract for the residual).

Layouts are pre-arranged on host so every matmul operand is a direct SBUF
slice: stationary tiles [128, 2, 128] (DoubleRow K-pair x M), moving tiles
[128, 2, tw]. s_hi stays resident in SBUF; s_lo spills to DRAM and streams
back during phase B (bandwidth is far under the PE time either way).
"""

import os
import sys
from contextlib import ExitStack

import numpy as np
import ml_dtypes

for _p in ("/opt/trn_rl_repo", "/root/.axon_site/_ro/trn_rl_repo"):
    if os.path.isdir(_p) and _p not in sys.path:
        sys.path.insert(0, _p)

import concourse.mybir as mybir
import concourse.tile as tile
from concourse.tile_rust import add_dep_helper
from concourse import bacc
from concourse.bass_utils import run_bass_kernel_spmd

FP32 = mybir.dt.float32
FP8 = mybir.dt.float8e4
ALU = mybir.AluOpType
AF = mybir.ActivationFunctionType
DR = mybir.MatmulPerfMode.DoubleRow
E4NP = ml_dtypes.float8_e4m3

E = 8            # experts == cores
C = 1024         # model dim
H0 = 2728        # ffn hidden dim
NKC = C // 256   # 4 DoubleRow contraction tiles over C
KH = 22          # 128-row h tiles (padded H)
NKH = KH // 2    # 11 DoubleRow contraction tiles over padded H
HP = KH * 128    # 2816
KC8 = C // 128   # 8 output c tiles
TT = 512         # max token tile (fp32 PSUM bank = 512 floats)
CAP_MAX = 2304   # per-launch token cap (SBUF budget); split into runs beyond

# fp8 scale factors. All hi/lo parts share their tensor's scale so the three
# split products accumulate in one PSUM chain.
SX = 16.0        # x*16: |x|<5.1 -> <82, x_lo ~0.4 (normal range)
SW = 1024.0      # w*1024: |w|<0.11 -> <113
SH = 8.0         # s*8: |s|<12 -> <96 (clip-safe), s_lo ~0.07
SA = SX * SW     # phase-A psum scale
SB = SH * SW     # phase-B psum scale

_CACHE = {}
_WCACHE = {}
LAST_RESULTS = None


def _token_tiles(cap):
    # sub-512 tiles go FIRST: the first matmul's DMA dependency is smaller,
    # so the PE starts (and ramps) earlier; tails kept >= 256 wide.
    widths = []
    left = cap
    while left > 640:
        widths.append(TT)
        left -= TT
    if left == 640:
        widths = [384, 256] + widths
    elif left > 0:
        widths = [left] + widths
    widths.sort()
    tiles = []
    off = 0
    for w in widths:
        tiles.append((off, w))
        off += w
    return tiles


def _split8(a):
    """a is pre-scaled fp32; return (hi, lo) e4m3 arrays at the same scale."""
    hi = np.clip(a, -240.0, 240.0).astype(E4NP)
    lo = (a - hi.astype(np.float32)).astype(E4NP)
    return hi, lo


def _build(cap):
    """Build + compile the SPMD program for `cap` tokens per core."""
    assert cap % 128 == 0
    tiles = _token_tiles(cap)
    last = len(tiles) - 1
    nc = bacc.Bacc("TRN2", target_bir_lowering=False, debug=False, num_devices=E)

    xh = nc.dram_tensor("xh", [NKC, 128, 2, cap], FP8, kind="ExternalInput").ap()
    xl = nc.dram_tensor("xl", [NKC, 128, 2, cap], FP8, kind="ExternalInput").ap()
    w1h = nc.dram_tensor("w1h", [KH, 128, NKC, 2, 128], FP8, kind="ExternalInput").ap()
    w1l = nc.dram_tensor("w1l", [KH, 128, NKC, 2, 128], FP8, kind="ExternalInput").ap()
    w3h = nc.dram_tensor("w3h", [KH, 128, NKC, 2, 128], FP8, kind="ExternalInput").ap()
    w3l = nc.dram_tensor("w3l", [KH, 128, NKC, 2, 128], FP8, kind="ExternalInput").ap()
    w2h = nc.dram_tensor("w2h", [KC8, 128, NKH, 2, 128], FP8, kind="ExternalInput").ap()
    w2l = nc.dram_tensor("w2l", [KC8, 128, NKH, 2, 128], FP8, kind="ExternalInput").ap()
    gsc = nc.dram_tensor("gsc", [cap], FP32, kind="ExternalInput").ap()
    yt = nc.dram_tensor("yt", [KC8, 128, cap], FP32, kind="ExternalOutput").ap()

    with tile.TileContext(nc) as tc, ExitStack() as top:
        dramp = top.enter_context(tc.tile_pool(name="dram", bufs=1, space="DRAM"))
        slo_dram = dramp.tile([128, NKH, 2, cap], FP8)

        shp = top.enter_context(tc.tile_pool(name="sres", bufs=1))
        s_hi = shp.tile([128, NKH, 2, cap], FP8)

        # w2 resident for the whole kernel; loads interleaved into phase A's
        # h-loop so they hide behind compute without starving startup DMA
        w2p = top.enter_context(tc.tile_pool(name="w2res", bufs=1))
        w2h_sb = [w2p.tile([128, NKH, 2, 128], FP8, tag=f"w2h{c}",
                           name=f"w2h_sb{c}") for c in range(KC8)]
        w2l_sb = [w2p.tile([128, NKH, 2, 128], FP8, tag=f"w2l{c}",
                           name=f"w2l_sb{c}") for c in range(KC8)]
        w2_loads = [(w2h_sb[c], w2h[c]) for c in range(KC8)] + \
                   [(w2l_sb[c], w2l[c]) for c in range(KC8)]

        anchor = None
        with ExitStack() as pha:
            xp = pha.enter_context(tc.tile_pool(name="xres", bufs=1))
            xh_sb = [xp.tile([128, 2, cap], FP8, tag=f"xh{k}", name=f"xh_sb{k}")
                     for k in range(NKC)]
            xl_sb = [xp.tile([128, 2, cap], FP8, tag=f"xl{k}", name=f"xl_sb{k}")
                     for k in range(NKC)]
            # startup-critical loads share the SYNC queue in consumption order
            wst = pha.enter_context(tc.tile_pool(name="wst", bufs=4))

            def walloc(h):
                return tuple(
                    wst.tile([128, NKC, 2, 128], FP8, tag=tg, name=f"{tg}_{h}")
                    for tg in ("w1h", "w1l", "w3h", "w3l"))

            def wload(h, wt):
                for t_, src in zip(wt, (w1h[h], w1l[h], w3h[h], w3l[h])):
                    nc.sync.dma_start(t_[:], src)

            npre = min(4, KH)
            w_cur = {h: walloc(h) for h in range(npre)}
            wload(0, w_cur[0])
            w_loaded = {0}
            for ti, (to, tw) in enumerate(tiles):
                for k in range(NKC):
                    nc.sync.dma_start(xh_sb[k][:, :, to:to + tw],
                                      xh[k][:, :, to:to + tw])
                    nc.sync.dma_start(xl_sb[k][:, :, to:to + tw],
                                      xl[k][:, :, to:to + tw])
                hh = ti + 1
                if ti <= 2 and hh in w_cur and hh not in w_loaded:
                    wload(hh, w_cur[hh])
                    w_loaded.add(hh)
            for hh in sorted(set(w_cur) - w_loaded):
                wload(hh, w_cur[hh])

            psa = pha.enter_context(tc.tile_pool(name="psA", bufs=3, space="PSUM"))
            stg = pha.enter_context(tc.tile_pool(name="stg", bufs=3))
            slop = pha.enter_context(tc.tile_pool(name="slo", bufs=4))

            def emit_ht(h, t, wt):
                nonlocal anchor
                w1h_t, w1l_t, w3h_t, w3l_t = wt
                to, tw = tiles[t]
                hd, blk = divmod(h, 2)
                p1 = psa.tile([128, TT], FP32, tag="p1", name=f"p1_{h}_{t}")
                p3 = psa.tile([128, TT], FP32, tag="p3", name=f"p3_{h}_{t}")
                for pp, wh_, wl_ in ((p1, w1h_t, w1l_t), (p3, w3h_t, w3l_t)):
                    n = 0
                    for xs_, ws_ in ((xh_sb, wh_), (xl_sb, wh_), (xh_sb, wl_)):
                        for kd in range(NKC):
                            anchor = nc.tensor.matmul(
                                pp[:, :tw], ws_[:, kd],
                                xs_[kd][:, :, to:to + tw],
                                start=(n == 0), stop=(n == 3 * NKC - 1),
                                perf_mode=DR)
                            n += 1
                sa = stg.tile([128, TT], FP32, tag="sa", name=f"sa{h}_{t}")
                nc.scalar.activation(sa[:, :tw], p1[:, :tw], AF.Silu,
                                     scale=1.0 / SA)
                t1 = stg.tile([128, TT], FP32, tag="t1", name=f"t1_{h}_{t}")
                acc = stg.tile([128, 1], FP32, tag="acc", name=f"acc{h}_{t}")
                nc.vector.affine_mul_reduce(t1[:, :tw], acc[:], p3[:, :tw],
                                            sa[:, :tw], SH / SA, 0.0)
                hi_sl = s_hi[:, hd, blk, to:to + tw]
                nc.scalar.activation(hi_sl, t1[:, :tw], AF.Copy)
                slo = slop.tile([128, TT], FP8, tag="slo", name=f"slo{h}_{t}")
                nc.vector.tensor_tensor(slo[:, :tw], t1[:, :tw], hi_sl,
                                        op=ALU.subtract)
                nc.gpsimd.dma_start(slo_dram[:, hd, blk, to:to + tw],
                                    slo[:, :tw])

            # emission: preloaded h over all tiles but the last, then the
            # last tile, then the streamed h with w2 prefetch interleaved
            for h in range(npre):
                for t in range(last):
                    emit_ht(h, t, w_cur[h])
            for h in range(npre):
                emit_ht(h, last, w_cur[h])
            for h in range(npre, KH):
                wt = walloc(h)
                wload(h, wt)
                w_cur[h] = wt
                j = h - npre
                if j < len(w2_loads):
                    dst, src = w2_loads[j]
                    w2dma = nc.gpsimd.dma_start(dst[:], src)
                    add_dep_helper(w2dma.ins, anchor.ins,
                                   reason="delay w2 prefetch")
                for t in range(len(tiles)):
                    emit_ht(h, t, w_cur[h])
            # two stragglers (KH - npre = 18 slots for 16 w2 loads) — none

        # ---- phase B: y = 3-term(s @ w2.T) * g ----
        with ExitStack() as phb:
            gbc = phb.enter_context(tc.tile_pool(name="gbc", bufs=1))
            g_sb = []
            for t, (to, tw) in enumerate(tiles):
                grow = gbc.tile([1, TT], FP32, tag="grow", name=f"grow{t}",
                                bufs=2)
                nc.gpsimd.dma_start(grow[0:1, :tw], gsc[to:to + tw])
                gt = gbc.tile([128, tw], FP32, tag=f"g{t}", name=f"g_sb{t}")
                nc.gpsimd.partition_broadcast(gt[:], grow[0:1, :tw])
                g_sb.append(gt)
            sin = phb.enter_context(tc.tile_pool(name="sin", bufs=2))
            psb = phb.enter_context(tc.tile_pool(name="psB", bufs=3, space="PSUM"))
            yp = phb.enter_context(tc.tile_pool(name="yst", bufs=4))
            b_order = sorted(range(len(tiles)),
                             key=lambda i: (i != 0, -tiles[i][1]))
            for t in b_order:
                to, tw = tiles[t]
                sl_t = sin.tile([128, NKH, 2, TT], FP8, tag="sl",
                                name=f"sl{t}")
                nc.scalar.dma_start(sl_t[:, :, :, :tw],
                                    slo_dram[:, :, :, to:to + tw])
                for c in range(KC8):
                    py = psb.tile([128, TT], FP32, tag="py", name=f"py{t}_{c}")
                    n = 0
                    for s_, w_ in ((s_hi, w2h_sb[c]), (None, w2h_sb[c]),
                                   (s_hi, w2l_sb[c])):
                        for hd in range(NKH):
                            mv = (sl_t[:, hd, :, :tw] if s_ is None
                                  else s_[:, hd, :, to:to + tw])
                            nc.tensor.matmul(py[:, :tw], w_[:, hd], mv,
                                             start=(n == 0),
                                             stop=(n == 3 * NKH - 1),
                                             perf_mode=DR)
                            n += 1
                    yb = yp.tile([128, TT], FP32, tag="y", name=f"yb{t}_{c}")
                    nc.vector.tensor_tensor(yb[:, :tw], py[:, :tw], g_sb[t][:],
                                            op=ALU.mult)
                    nc.sync.dma_start(yt[c, :, to:to + tw], yb[:, :tw])

    nc.compile()
    return nc


def _prep_weights(gw, w1, w2, w3):
    """Quantize + arrange all per-expert weight tensors (host, cached)."""
    wmaps = []
    for e in range(E):
        m = {}
        for nm, w, out_shape in (("w1", w1[e], None), ("w3", w3[e], None)):
            wt = np.zeros((HP, C), np.float32)
            wt[:H0] = w
            hi, lo = _split8(wt * SW)
            # [HP, C] -> [KH, 128m, NKC, 2, 128p] -> [KH, 128p, NKC, 2, 128m]
            for part, arr in (("h", hi), ("l", lo)):
                a = arr.reshape(KH, 128, NKC, 2, 128).transpose(0, 4, 2, 3, 1)
                m[nm + part] = np.ascontiguousarray(a)
        wt = np.zeros((C, HP), np.float32)
        wt[:, :H0] = w2[e]
        hi, lo = _split8(wt * SW)
        # [C, HP] -> [KC8, 128m, NKH, 2, 128p] -> [KC8, 128p, NKH, 2, 128m]
        for part, arr in (("h", hi), ("l", lo)):
            a = arr.reshape(KC8, 128, NKH, 2, 128).transpose(0, 4, 2, 3, 1)
            m["w2" + part] = np.ascontiguousarray(a)
        wmaps.append(m)
    return wmaps


def kernel(x, gate_w, w1, w2, w3, top_k):
    global LAST_RESULTS
    x = np.asarray(x, dtype=np.float32)
    gw = np.asarray(gate_w, dtype=np.float32)
    w1 = np.asarray(w1, dtype=np.float32)
    w2 = np.asarray(w2, dtype=np.float32)
    w3 = np.asarray(w3, dtype=np.float32)
    assert int(np.asarray(top_k)) == 2
    Bb, T, Cc = x.shape
    N = Bb * T
    assert Cc == C and w1.shape == (E, H0, C)

    xf = np.ascontiguousarray(x.reshape(N, C))
    # Router on host (exact fp32): top-2 selection + softmax combine weights.
    logits = xf @ gw.T
    order = np.argsort(-logits, axis=1, kind="stable")[:, :2]
    vals = np.take_along_axis(logits, order, axis=1)
    sw = np.exp(vals - vals.max(axis=1, keepdims=True))
    sw /= sw.sum(axis=1, keepdims=True)
    tok, gtok = [], []
    for e in range(E):
        sel = order == e
        idx = np.nonzero(sel.any(axis=1))[0]
        tok.append(idx)
        gtok.append(sw[sel].astype(np.float32))

    key = id(gate_w) if not isinstance(gate_w, np.ndarray) else id(w1)
    wm = _WCACHE.get(key)
    if wm is None:
        wm = _prep_weights(gw, w1, w2, w3)
        _WCACHE.clear()
        _WCACHE[key] = wm

    # quantize x once (full token set), dispatch indexes the fp8 arrays
    xs = xf * SX
    xh_full, xl_full = _split8(xs)

    out = np.zeros((N, C), np.float32)
    nchunk = (max(t.size for t in tok) + CAP_MAX - 1) // CAP_MAX
    for ci in range(nchunk):
        tokc, gc = [], []
        for e in range(E):
            lo_ = (ci * tok[e].size) // nchunk
            hi_ = ((ci + 1) * tok[e].size) // nchunk
            tokc.append(tok[e][lo_:hi_])
            gc.append(gtok[e][lo_:hi_])
        cap = max(TT, ((max(t.size for t in tokc) + 127) // 128) * 128)
        if cap not in _CACHE:
            _CACHE[cap] = _build(cap)
        nc = _CACHE[cap]
        in_maps = []
        for e in range(E):
            idx = tokc[e]
            n = idx.size
            im = dict(wm[e])
            for nm, full in (("xh", xh_full), ("xl", xl_full)):
                xe = np.zeros((cap, C), E4NP)
                xe[:n] = full[idx]
                # [cap, C] -> [NKC, 2, 128p, cap] with c = kd*256+i*128+p
                a = xe.T.reshape(NKC, 2, 128, cap).transpose(0, 2, 1, 3)
                im[nm] = np.ascontiguousarray(a)
            g = np.zeros(cap, np.float32)
            g[:n] = gc[e] / SB
            im["gsc"] = g
            in_maps.append(im)

        res = run_bass_kernel_spmd(nc, in_maps, core_ids=list(range(E)))
        LAST_RESULTS = res

        for e in range(E):
            idx = tokc[e]
            n = idx.size
            ye = res.results[e]["yt"].reshape(C, cap).T
            out[idx] += ye[:n]
    return out.reshape(Bb, T, C)


# revision 46
# speedup vs baseline: 1.3549x; 1.0451x over previous
"""Expert-parallel MoE layer for Trainium2 (Bass/Tile, 8 NeuronCores).

Strategy (hardcoded for B=4, T=2048, C=1024, E=8, H=2728, top_k=2):
  - Expert-parallel: core e owns expert e's weights (w1/w2/w3[e]).
  - Host computes the router (top-2 ids AND the softmax combine weights in
    exact fp32) and performs the all-to-all token dispatch/combine as the
    shard/unshard step. The per-token gate weight rides in as a small fp32
    vector, pre-scaled by the fp8 scale factors.
  - Each core computes the full expert FFN y = (silu(x@w1.T) * (x@w3.T))
    @ w2.T * g for its tokens, entirely in fp8-e4m3 DoubleRow matmuls
    (K=256 per instruction, 0.5 cycles/column — 2x the fp32r MAC rate).

Precision: every GEMM uses a 3-term hi/lo split, all at one shared scale so
the three products accumulate in a single PSUM chain:
    A@W ~= Ah@Wh + Al@Wh + Ah@Wl      (drops only the Al@Wl term, ~7e-4)
with Ah = e4m3(A*S), Al = e4m3(A*S - Ah). End-to-end rel err ~2e-3 vs the
2e-2 gate. x/w splits happen on host; the phase-A output s is split on
device (ACT copy for the hi part, DVE subtract for the residual).

Layouts are pre-arranged on host so every matmul operand is a direct SBUF
slice: stationary tiles [128, 2, 128] (DoubleRow K-pair x M), moving tiles
[128, 2, tw]. s_hi stays resident in SBUF; s_lo spills to DRAM and streams
back during phase B (bandwidth is far under the PE time either way).
"""

import os
import sys
from contextlib import ExitStack

import numpy as np
import ml_dtypes

for _p in ("/opt/trn_rl_repo", "/root/.axon_site/_ro/trn_rl_repo"):
    if os.path.isdir(_p) and _p not in sys.path:
        sys.path.insert(0, _p)

import concourse.mybir as mybir
import concourse.tile as tile
from concourse.tile_rust import add_dep_helper
from concourse import bacc
from concourse.bass_utils import run_bass_kernel_spmd

FP32 = mybir.dt.float32
FP8 = mybir.dt.float8e4
ALU = mybir.AluOpType
AF = mybir.ActivationFunctionType
DR = mybir.MatmulPerfMode.DoubleRow
E4NP = ml_dtypes.float8_e4m3

E = 8            # experts == cores
C = 1024         # model dim
H0 = 2728        # ffn hidden dim
NKC = C // 256   # 4 DoubleRow contraction tiles over C
KH = 22          # 128-row h tiles (padded H)
NKH = KH // 2    # 11 DoubleRow contraction tiles over padded H
HP = KH * 128    # 2816
KC8 = C // 128   # 8 output c tiles
TT = 512         # max token tile (fp32 PSUM bank = 512 floats)
CAP_MAX = 2304   # per-launch token cap (SBUF budget); split into runs beyond

# fp8 scale factors. All hi/lo parts share their tensor's scale so the three
# split products accumulate in one PSUM chain.
SX = 16.0        # x*16: |x|<5.1 -> <82, x_lo ~0.4 (normal range)
SW = 1024.0      # w*1024: |w|<0.11 -> <113
SH = 8.0         # s*8: |s|<12 -> <96 (clip-safe), s_lo ~0.07
SA = SX * SW     # phase-A psum scale
SB = SH * SW     # phase-B psum scale

_CACHE = {}
_WCACHE = {}
LAST_RESULTS = None

# startup-schedule knobs (fixed by a TimelineSim sweep)
XH_SCALAR = (1, 3)      # xh tile indices that ride the ACT queue
W0SPLIT = True          # split whi[0] into w1/w3 halves around xh0
RAMP = (256, 384)       # leading token-tile widths


def _token_tiles(cap):
    # sub-512 tiles go FIRST (ascending): the first matmuls' DMA
    # dependencies are smaller, so the PE starts (and ramps) earlier. A
    # 128-wide leader is fine at fp8-DR (no narrow-tile rate penalty).
    ramp = list(RAMP)
    while sum(ramp) > max(0, cap - 256) and len(ramp) > 1:
        ramp.pop()
    widths = list(ramp)
    left = cap - sum(ramp)
    if left % TT:
        widths.append(left % TT)
    widths += [TT] * (left // TT)
    widths.sort()
    tiles = []
    off = 0
    for w in widths:
        tiles.append((off, w))
        off += w
    return tiles


def _split8(a):
    """a is pre-scaled fp32; return (hi, lo) e4m3 arrays at the same scale."""
    hi = np.clip(a, -240.0, 240.0).astype(E4NP)
    lo = (a - hi.astype(np.float32)).astype(E4NP)
    return hi, lo


def _build(cap):
    """Build + compile the SPMD program for `cap` tokens per core."""
    assert cap % 128 == 0
    tiles = _token_tiles(cap)
    last = len(tiles) - 1
    nc = bacc.Bacc("TRN2", target_bir_lowering=False, debug=False, num_devices=E)

    xh = nc.dram_tensor("xh", [128, NKC, 2, cap], FP8, kind="ExternalInput").ap()
    xl = nc.dram_tensor("xl", [128, NKC, 2, cap], FP8, kind="ExternalInput").ap()
    # w1h+w3h (resp. w1l+w3l) fused per h-tile: one DMA instead of two
    # (fixed cost per DMA dominates these small transfers)
    whi = nc.dram_tensor("whi", [KH, 128, 2, NKC, 2, 128], FP8, kind="ExternalInput").ap()
    wlo = nc.dram_tensor("wlo", [KH, 128, 2, NKC, 2, 128], FP8, kind="ExternalInput").ap()
    w2h = nc.dram_tensor("w2h", [KC8, 128, NKH, 2, 128], FP8, kind="ExternalInput").ap()
    w2l = nc.dram_tensor("w2l", [KC8, 128, NKH, 2, 128], FP8, kind="ExternalInput").ap()
    gsc = nc.dram_tensor("gsc", [cap], FP32, kind="ExternalInput").ap()
    yt = nc.dram_tensor("yt", [KC8, 128, cap], FP32, kind="ExternalOutput").ap()

    with tile.TileContext(nc) as tc, ExitStack() as top:
        dramp = top.enter_context(tc.tile_pool(name="dram", bufs=1, space="DRAM"))
        # one scratch tensor per token tile so the phase-B reload of tile t
        # only depends on tile t's writes, not the whole phase A
        ntile = len(tiles)
        slo_dram = [dramp.tile([128, NKH, 2, TT], FP8, tag=f"slo{t}",
                               name=f"slo_dram{t}")
                    for t in range(ntile)]

        shp = top.enter_context(tc.tile_pool(name="sres", bufs=1))
        s_hi = shp.tile([128, NKH, 2, cap], FP8)

        # w2 resident for the whole kernel; loads interleaved into phase A's
        # h-loop so they hide behind compute without starving startup DMA
        w2p = top.enter_context(tc.tile_pool(name="w2res", bufs=1))
        w2h_sb = [w2p.tile([128, NKH, 2, 128], FP8, tag=f"w2h{c}",
                           name=f"w2h_sb{c}") for c in range(KC8)]
        w2l_sb = [w2p.tile([128, NKH, 2, 128], FP8, tag=f"w2l{c}",
                           name=f"w2l_sb{c}") for c in range(KC8)]
        w2_loads = [(w2h_sb[c], w2h[c]) for c in range(KC8)] + \
                   [(w2l_sb[c], w2l[c]) for c in range(KC8)]

        gbc = top.enter_context(tc.tile_pool(name="gbc", bufs=1))
        g_sb = []

        def emit_g():
            # gate-weight rows: tiny loads + partition broadcasts on the
            # SWDGE queue, emitted mid-phase-A where that queue has slack —
            # NOT at the phase boundary, where they'd sit behind the s_lo
            # write backlog and stall the first y-multiplies
            for t, (to, tw) in enumerate(tiles):
                grow = gbc.tile([1, TT], FP32, tag="grow", name=f"grow{t}",
                                bufs=2)
                nc.gpsimd.dma_start(grow[0:1, :tw], gsc[to:to + tw])
                gt = gbc.tile([128, tw], FP32, tag=f"g{t}", name=f"g_sb{t}")
                nc.gpsimd.partition_broadcast(gt[:], grow[0:1, :tw])
                g_sb.append(gt)

        anchor = None
        with ExitStack() as pha:
            xp = pha.enter_context(tc.tile_pool(name="xres", bufs=1))
            xh_sb = xp.tile([128, NKC, 2, cap], FP8, name="xh_sb")
            xl_sb = xp.tile([128, NKC, 2, cap], FP8, name="xl_sb")
            wst = pha.enter_context(tc.tile_pool(name="wst", bufs=4))

            def walloc(h):
                return (
                    wst.tile([128, 2, NKC, 2, 128], FP8, tag="whi", name=f"whi_{h}"),
                    wst.tile([128, 2, NKC, 2, 128], FP8, tag="wlo", name=f"wlo_{h}"),
                )

            # startup streams split across the two free queues in exact
            # consumption order (per-queue DMA processing is FIFO): SYNC
            # carries the hi parts (consumed first in every chain) + xh;
            # the gpsimd/SWDGE queue carries xl + the fused lo parts. The
            # ACT queue must stay empty here: each DMA on it would occupy
            # the ACT sequencer ~1.3us and push the silu/quantize chain
            # (and with it PSUM recycling) out by that much. x rides ahead
            # of the h>=1 weights: each xh tile is consumed ~1us after the
            # previous, while w[h] only gates the next 11us-long h-sweep.
            def wload_hi(h, wt):
                nc.sync.dma_start(wt[0][:], whi[h])

            def wload_lo(h, wt):
                nc.gpsimd.dma_start(wt[1][:], wlo[h])

            npre = min(4, KH)
            w_cur = {h: walloc(h) for h in range(npre)}
            # h0's hi weights optionally split in two: the first chain only
            # needs the w1 half, so it rides ahead of xh0, w3 follows
            if W0SPLIT:
                nc.sync.dma_start(w_cur[0][0][:, 0], whi[0][:, 0])
            else:
                wload_hi(0, w_cur[0])
            for ti, (to, tw) in enumerate(tiles):
                # some xh tiles ride the ACT queue: a third startup channel
                # (ACT has no compute until the first silu lands, well
                # after these triggers retire)
                xq = nc.scalar if ti in XH_SCALAR else nc.sync
                xq.dma_start(xh_sb[:, :, :, to:to + tw],
                             xh[:, :, :, to:to + tw])
                nc.gpsimd.dma_start(xl_sb[:, :, :, to:to + tw],
                                    xl[:, :, :, to:to + tw])
                if ti == 0:
                    if W0SPLIT:
                        nc.sync.dma_start(w_cur[0][0][:, 1], whi[0][:, 1])
                    wload_lo(0, w_cur[0])
            # h>=1 weights trail the full x stream on both queues: each xh
            # tile is consumed within ~1-2us, while w[h] only gates the
            # next 11us-long h-sweep
            for h in range(1, npre):
                wload_hi(h, w_cur[h])
                wload_lo(h, w_cur[h])

            psa = pha.enter_context(tc.tile_pool(name="psA", bufs=2, space="PSUM"))
            stg = pha.enter_context(tc.tile_pool(name="stg", bufs=3))
            # deep staging: s_lo DMA-out rides the busy SWDGE queue, so the
            # writes may lag the compute by several (h,t) groups
            slop = pha.enter_context(tc.tile_pool(name="slo", bufs=12))

            def emit_ht(h, t, wt):
                nonlocal anchor
                whi_t, wlo_t = wt
                to, tw = tiles[t]
                hd, blk = divmod(h, 2)
                p1 = psa.tile([128, TT], FP32, tag="p1", name=f"p1_{h}_{t}")
                p3 = psa.tile([128, TT], FP32, tag="p3", name=f"p3_{h}_{t}")
                for pp, wh_, wl_ in ((p1, whi_t[:, 0], wlo_t[:, 0]),
                                     (p3, whi_t[:, 1], wlo_t[:, 1])):
                    n = 0
                    for xs_, ws_ in ((xh_sb, wh_), (xl_sb, wh_), (xh_sb, wl_)):
                        for kd in range(NKC):
                            anchor = nc.tensor.matmul(
                                pp[:, :tw], ws_[:, kd],
                                xs_[:, kd, :, to:to + tw],
                                start=(n == 0), stop=(n == 3 * NKC - 1),
                                perf_mode=DR)
                            n += 1
                sa = stg.tile([128, TT], FP32, tag="sa", name=f"sa{h}_{t}")
                nc.scalar.activation(sa[:, :tw], p1[:, :tw], AF.Silu,
                                     scale=1.0 / SA)
                t1 = stg.tile([128, TT], FP32, tag="t1", name=f"t1_{h}_{t}")
                acc = stg.tile([128, 1], FP32, tag="acc", name=f"acc{h}_{t}")
                nc.vector.affine_mul_reduce(t1[:, :tw], acc[:], p3[:, :tw],
                                            sa[:, :tw], SH / SA, 0.0)
                hi_sl = s_hi[:, hd, blk, to:to + tw]
                nc.scalar.activation(hi_sl, t1[:, :tw], AF.Copy)
                slo = slop.tile([128, TT], FP8, tag="slo", name=f"slo{h}_{t}")
                nc.vector.tensor_tensor(slo[:, :tw], t1[:, :tw], hi_sl,
                                        op=ALU.subtract)
                nc.gpsimd.dma_start(slo_dram[t][:, hd, blk, :tw],
                                    slo[:, :tw])

            # emission: preloaded h-levels h-major, ramped tiles ascending
            for h in range(npre):
                for t in range(len(tiles)):
                    emit_ht(h, t, w_cur[h])
            for h in range(npre, KH):
                wt = walloc(h)
                wload_hi(h, wt)
                wload_lo(h, wt)
                w_cur[h] = wt
                j = h - npre
                if j < len(w2_loads):
                    dst, src = w2_loads[j]
                    w2dma = nc.gpsimd.dma_start(dst[:], src)
                    add_dep_helper(w2dma.ins, anchor.ins,
                                   reason="delay w2 prefetch")
                if h == 6:
                    emit_g()
                for t in range(len(tiles)):
                    emit_ht(h, t, w_cur[h])
            # two stragglers (KH - npre = 18 slots for 16 w2 loads) — none

        # ---- phase B: y = 3-term(s @ w2.T) * g ----
        with ExitStack() as phb:
            sin = phb.enter_context(tc.tile_pool(name="sin", bufs=2))
            psb = phb.enter_context(tc.tile_pool(name="psB", bufs=3, space="PSUM"))
            yp = phb.enter_context(tc.tile_pool(name="yst", bufs=4))
            # big tiles in the middle; the smallest tile LAST so the final
            # y writeback (which trails the last matmul) is the shortest
            b_order = sorted(range(len(tiles)),
                             key=lambda i: (-tiles[i][1], i))
            b_order = b_order[:-1] + [b_order[-1]]
            sm = min(range(len(tiles)), key=lambda i: tiles[i][1])
            b_order = [i for i in b_order if i != sm] + [sm]
            # the first tile's s_lo reload rides SYNC (idle through phase A,
            # and the wait on that tile's writes resolves ~10us before the
            # A/B boundary); later tiles go per-tile on the ACT queue, which
            # frees up right at the boundary
            for bi, t in enumerate(b_order):
                to, tw = tiles[t]
                sl_t = sin.tile([128, NKH, 2, TT], FP8, tag="sl",
                                name=f"sl_sb{t}")
                q = nc.sync if bi == 0 else nc.scalar
                q.dma_start(sl_t[:, :, :, :tw], slo_dram[t][:, :, :, :tw])
                for c in range(KC8):
                    py = psb.tile([128, TT], FP32, tag="py", name=f"py{t}_{c}")
                    n = 0
                    # the s_lo group goes LAST so the chain can start before
                    # the reload DMA of this tile's s_lo has landed
                    for s_, w_ in ((s_hi, w2h_sb[c]), (s_hi, w2l_sb[c]),
                                   (None, w2h_sb[c])):
                        for hd in range(NKH):
                            mv = (sl_t[:, hd, :, :tw] if s_ is None
                                  else s_[:, hd, :, to:to + tw])
                            nc.tensor.matmul(py[:, :tw], w_[:, hd], mv,
                                             start=(n == 0),
                                             stop=(n == 3 * NKH - 1),
                                             perf_mode=DR)
                            n += 1
                    yb = yp.tile([128, TT], FP32, tag="y", name=f"yb{t}_{c}")
                    nc.vector.tensor_tensor(yb[:, :tw], py[:, :tw], g_sb[t][:],
                                            op=ALU.mult)
                    # alternate writeback queues: halves the per-queue y
                    # rate so the final transfer doesn't trail the compute
                    yq = nc.sync if c % 2 == 0 else nc.scalar
                    yq.dma_start(yt[c, :, to:to + tw], yb[:, :tw])

    nc.compile()
    return nc


def _prep_weights(gw, w1, w2, w3):
    """Quantize + arrange all per-expert weight tensors (host, cached)."""
    wmaps = []
    for e in range(E):
        m = {}
        his, los = {}, {}
        for nm, w in (("w1", w1[e]), ("w3", w3[e])):
            wt = np.zeros((HP, C), np.float32)
            wt[:H0] = w
            hi, lo = _split8(wt * SW)
            # [HP, C] -> [KH, 128m, NKC, 2, 128p] -> [KH, 128p, NKC, 2, 128m]
            his[nm] = hi.reshape(KH, 128, NKC, 2, 128).transpose(0, 4, 2, 3, 1)
            los[nm] = lo.reshape(KH, 128, NKC, 2, 128).transpose(0, 4, 2, 3, 1)
        m["whi"] = np.ascontiguousarray(
            np.stack([his["w1"], his["w3"]], axis=2))
        m["wlo"] = np.ascontiguousarray(
            np.stack([los["w1"], los["w3"]], axis=2))
        wt = np.zeros((C, HP), np.float32)
        wt[:, :H0] = w2[e]
        hi, lo = _split8(wt * SW)
        # [C, HP] -> [KC8, 128m, NKH, 2, 128p] -> [KC8, 128p, NKH, 2, 128m]
        for part, arr in (("h", hi), ("l", lo)):
            a = arr.reshape(KC8, 128, NKH, 2, 128).transpose(0, 4, 2, 3, 1)
            m["w2" + part] = np.ascontiguousarray(a)
        wmaps.append(m)
    return wmaps


def kernel(x, gate_w, w1, w2, w3, top_k):
    global LAST_RESULTS
    x = np.asarray(x, dtype=np.float32)
    gw = np.asarray(gate_w, dtype=np.float32)
    w1 = np.asarray(w1, dtype=np.float32)
    w2 = np.asarray(w2, dtype=np.float32)
    w3 = np.asarray(w3, dtype=np.float32)
    assert int(np.asarray(top_k)) == 2
    Bb, T, Cc = x.shape
    N = Bb * T
    assert Cc == C and w1.shape == (E, H0, C)

    xf = np.ascontiguousarray(x.reshape(N, C))
    # Router on host (exact fp32): top-2 selection + softmax combine weights.
    logits = xf @ gw.T
    order = np.argsort(-logits, axis=1, kind="stable")[:, :2]
    vals = np.take_along_axis(logits, order, axis=1)
    sw = np.exp(vals - vals.max(axis=1, keepdims=True))
    sw /= sw.sum(axis=1, keepdims=True)
    tok, gtok = [], []
    for e in range(E):
        sel = order == e
        idx = np.nonzero(sel.any(axis=1))[0]
        tok.append(idx)
        gtok.append(sw[sel].astype(np.float32))

    key = id(gate_w) if not isinstance(gate_w, np.ndarray) else id(w1)
    wm = _WCACHE.get(key)
    if wm is None:
        wm = _prep_weights(gw, w1, w2, w3)
        _WCACHE.clear()
        _WCACHE[key] = wm

    # quantize x once (full token set), dispatch indexes the fp8 arrays
    xs = xf * SX
    xh_full, xl_full = _split8(xs)

    out = np.zeros((N, C), np.float32)
    nchunk = (max(t.size for t in tok) + CAP_MAX - 1) // CAP_MAX
    for ci in range(nchunk):
        tokc, gc = [], []
        for e in range(E):
            lo_ = (ci * tok[e].size) // nchunk
            hi_ = ((ci + 1) * tok[e].size) // nchunk
            tokc.append(tok[e][lo_:hi_])
            gc.append(gtok[e][lo_:hi_])
        cap = max(TT, ((max(t.size for t in tokc) + 127) // 128) * 128)
        if cap not in _CACHE:
            _CACHE[cap] = _build(cap)
        nc = _CACHE[cap]
        in_maps = []
        for e in range(E):
            idx = tokc[e]
            n = idx.size
            im = dict(wm[e])
            for nm, full in (("xh", xh_full), ("xl", xl_full)):
                xe = np.zeros((cap, C), E4NP)
                xe[:n] = full[idx]
                # [cap, C] -> [128p, NKC, 2, cap] with c = kd*256+i*128+p
                a = xe.T.reshape(NKC, 2, 128, cap).transpose(2, 0, 1, 3)
                im[nm] = np.ascontiguousarray(a)
            g = np.zeros(cap, np.float32)
            g[:n] = gc[e] / SB
            im["gsc"] = g
            in_maps.append(im)

        res = run_bass_kernel_spmd(nc, in_maps, core_ids=list(range(E)))
        LAST_RESULTS = res

        for e in range(E):
            idx = tokc[e]
            n = idx.size
            ye = res.results[e]["yt"].reshape(C, cap).T
            out[idx] += ye[:n]
    return out.reshape(Bb, T, C)


# revision 49
# speedup vs baseline: 1.3580x; 1.0023x over previous
"""Expert-parallel MoE layer for Trainium2 (Bass/Tile, 8 NeuronCores).

Strategy (hardcoded for B=4, T=2048, C=1024, E=8, H=2728, top_k=2):
  - Expert-parallel: core e owns expert e's weights (w1/w2/w3[e]).
  - Host computes the router (top-2 ids AND the softmax combine weights in
    exact fp32) and performs the all-to-all token dispatch/combine as the
    shard/unshard step. The per-token gate weight rides in as a small fp32
    vector, pre-scaled by the fp8 scale factors.
  - Each core computes the full expert FFN y = (silu(x@w1.T) * (x@w3.T))
    @ w2.T * g for its tokens, entirely in fp8-e4m3 DoubleRow matmuls
    (K=256 per instruction, 0.5 cycles/column — 2x the fp32r MAC rate).

Precision: every GEMM uses a 3-term hi/lo split, all at one shared scale so
the three products accumulate in a single PSUM chain:
    A@W ~= Ah@Wh + Al@Wh + Ah@Wl      (drops only the Al@Wl term, ~7e-4)
with Ah = e4m3(A*S), Al = e4m3(A*S - Ah). End-to-end rel err ~2e-3 vs the
2e-2 gate. x/w splits happen on host; the phase-A output s is split on
device (ACT copy for the hi part, DVE subtract for the residual).

Layouts are pre-arranged on host so every matmul operand is a direct SBUF
slice: stationary tiles [128, 2, 128] (DoubleRow K-pair x M), moving tiles
[128, 2, tw]. s_hi stays resident in SBUF; s_lo spills to DRAM and streams
back during phase B (bandwidth is far under the PE time either way).
"""

import os
import sys
from contextlib import ExitStack

import numpy as np
import ml_dtypes

for _p in ("/opt/trn_rl_repo", "/root/.axon_site/_ro/trn_rl_repo"):
    if os.path.isdir(_p) and _p not in sys.path:
        sys.path.insert(0, _p)

import concourse.mybir as mybir
import concourse.tile as tile
from concourse.tile_rust import add_dep_helper
from concourse import bacc
from concourse.bass_utils import run_bass_kernel_spmd

FP32 = mybir.dt.float32
FP8 = mybir.dt.float8e4
ALU = mybir.AluOpType
AF = mybir.ActivationFunctionType
DR = mybir.MatmulPerfMode.DoubleRow
E4NP = ml_dtypes.float8_e4m3

E = 8            # experts == cores
C = 1024         # model dim
H0 = 2728        # ffn hidden dim
NKC = C // 256   # 4 DoubleRow contraction tiles over C
KH = 22          # 128-row h tiles (padded H)
NKH = KH // 2    # 11 DoubleRow contraction tiles over padded H
HP = KH * 128    # 2816
KC8 = C // 128   # 8 output c tiles
TT = 512         # max token tile (fp32 PSUM bank = 512 floats)
CAP_MAX = 2304   # per-launch token cap (SBUF budget); split into runs beyond

# fp8 scale factors. All hi/lo parts share their tensor's scale so the three
# split products accumulate in one PSUM chain.
SX = 16.0        # x*16: |x|<5.1 -> <82, x_lo ~0.4 (normal range)
SW = 1024.0      # w*1024: |w|<0.11 -> <113
SH = 8.0         # s*8: |s|<12 -> <96 (clip-safe), s_lo ~0.07
SA = SX * SW     # phase-A psum scale
SB = SH * SW     # phase-B psum scale

_CACHE = {}
_WCACHE = {}
LAST_RESULTS = None

# startup-schedule knobs (fixed by a TimelineSim sweep)
XH_SCALAR = (1, 3)      # xh tile indices that ride the ACT queue
W0SPLIT = True          # split whi[0] into w1/w3 halves around xh0
RAMP = (256, 384)       # leading token-tile widths


def _token_tiles(cap):
    # sub-512 tiles go FIRST (ascending): the first matmuls' DMA
    # dependencies are smaller, so the PE starts (and ramps) earlier. A
    # 128-wide leader is fine at fp8-DR (no narrow-tile rate penalty).
    ramp = list(RAMP)
    while sum(ramp) > max(0, cap - 256) and len(ramp) > 1:
        ramp.pop()
    widths = list(ramp)
    left = cap - sum(ramp)
    if left % TT:
        widths.append(left % TT)
    widths += [TT] * (left // TT)
    widths.sort()
    tiles = []
    off = 0
    for w in widths:
        tiles.append((off, w))
        off += w
    return tiles


def _split8(a):
    """a is pre-scaled fp32; return (hi, lo) e4m3 arrays at the same scale."""
    hi = np.clip(a, -240.0, 240.0).astype(E4NP)
    lo = (a - hi.astype(np.float32)).astype(E4NP)
    return hi, lo


def _build(cap):
    """Build + compile the SPMD program for `cap` tokens per core."""
    assert cap % 128 == 0
    tiles = _token_tiles(cap)
    last = len(tiles) - 1
    nc = bacc.Bacc("TRN2", target_bir_lowering=False, debug=False, num_devices=E)

    xh = nc.dram_tensor("xh", [128, NKC, 2, cap], FP8, kind="ExternalInput").ap()
    xl = nc.dram_tensor("xl", [128, NKC, 2, cap], FP8, kind="ExternalInput").ap()
    # w1h+w3h (resp. w1l+w3l) fused per h-tile: one DMA instead of two
    # (fixed cost per DMA dominates these small transfers)
    whi = nc.dram_tensor("whi", [KH, 128, 2, NKC, 2, 128], FP8, kind="ExternalInput").ap()
    wlo = nc.dram_tensor("wlo", [KH, 128, 2, NKC, 2, 128], FP8, kind="ExternalInput").ap()
    w2h = nc.dram_tensor("w2h", [KC8, 128, NKH, 2, 128], FP8, kind="ExternalInput").ap()
    w2l = nc.dram_tensor("w2l", [KC8, 128, NKH, 2, 128], FP8, kind="ExternalInput").ap()
    gsc = nc.dram_tensor("gsc", [cap], FP32, kind="ExternalInput").ap()
    yt = nc.dram_tensor("yt", [KC8, 128, cap], FP32, kind="ExternalOutput").ap()

    with tile.TileContext(nc) as tc, ExitStack() as top:
        dramp = top.enter_context(tc.tile_pool(name="dram", bufs=1, space="DRAM"))
        # one scratch tensor per token tile so the phase-B reload of tile t
        # only depends on tile t's writes, not the whole phase A
        ntile = len(tiles)
        slo_dram = [dramp.tile([128, NKH, 2, TT], FP8, tag=f"slo{t}",
                               name=f"slo_dram{t}")
                    for t in range(ntile)]

        shp = top.enter_context(tc.tile_pool(name="sres", bufs=1))
        s_hi = shp.tile([128, NKH, 2, cap], FP8)

        # w2 resident for the whole kernel; loads interleaved into phase A's
        # h-loop so they hide behind compute without starving startup DMA
        w2p = top.enter_context(tc.tile_pool(name="w2res", bufs=1))
        w2h_sb = [w2p.tile([128, NKH, 2, 128], FP8, tag=f"w2h{c}",
                           name=f"w2h_sb{c}") for c in range(KC8)]
        w2l_sb = [w2p.tile([128, NKH, 2, 128], FP8, tag=f"w2l{c}",
                           name=f"w2l_sb{c}") for c in range(KC8)]
        w2_loads = [(w2h_sb[c], w2h[c]) for c in range(KC8)] + \
                   [(w2l_sb[c], w2l[c]) for c in range(KC8)]

        gbc = top.enter_context(tc.tile_pool(name="gbc", bufs=1))
        g_sb = []

        def emit_g():
            # gate-weight rows: tiny loads + partition broadcasts on the
            # SWDGE queue, emitted mid-phase-A where that queue has slack —
            # NOT at the phase boundary, where they'd sit behind the s_lo
            # write backlog and stall the first y-multiplies
            for t, (to, tw) in enumerate(tiles):
                grow = gbc.tile([1, TT], FP32, tag="grow", name=f"grow{t}",
                                bufs=2)
                nc.gpsimd.dma_start(grow[0:1, :tw], gsc[to:to + tw])
                gt = gbc.tile([128, tw], FP32, tag=f"g{t}", name=f"g_sb{t}")
                nc.gpsimd.partition_broadcast(gt[:], grow[0:1, :tw])
                g_sb.append(gt)

        # phase B's PSUM pool is allocated up front so it lands in banks
        # disjoint from phase A's — otherwise B's first chain waits ~1us
        # for A's tail to release a recycled bank
        psb = top.enter_context(tc.tile_pool(name="psB", bufs=3, space="PSUM"))
        anchor = None
        with ExitStack() as pha:
            xp = pha.enter_context(tc.tile_pool(name="xres", bufs=1))
            xh_sb = xp.tile([128, NKC, 2, cap], FP8, name="xh_sb")
            xl_sb = xp.tile([128, NKC, 2, cap], FP8, name="xl_sb")
            wst = pha.enter_context(tc.tile_pool(name="wst", bufs=4))

            def walloc(h):
                return (
                    wst.tile([128, 2, NKC, 2, 128], FP8, tag="whi", name=f"whi_{h}"),
                    wst.tile([128, 2, NKC, 2, 128], FP8, tag="wlo", name=f"wlo_{h}"),
                )

            # startup streams split across the two free queues in exact
            # consumption order (per-queue DMA processing is FIFO): SYNC
            # carries the hi parts (consumed first in every chain) + xh;
            # the gpsimd/SWDGE queue carries xl + the fused lo parts. The
            # ACT queue must stay empty here: each DMA on it would occupy
            # the ACT sequencer ~1.3us and push the silu/quantize chain
            # (and with it PSUM recycling) out by that much. x rides ahead
            # of the h>=1 weights: each xh tile is consumed ~1us after the
            # previous, while w[h] only gates the next 11us-long h-sweep.
            def wload_hi(h, wt):
                nc.sync.dma_start(wt[0][:], whi[h])

            def wload_lo(h, wt):
                nc.gpsimd.dma_start(wt[1][:], wlo[h])

            npre = min(4, KH)
            w_cur = {h: walloc(h) for h in range(npre)}
            # h0's hi weights optionally split in two: the first chain only
            # needs the w1 half, so it rides ahead of xh0, w3 follows
            if W0SPLIT:
                nc.sync.dma_start(w_cur[0][0][:, 0], whi[0][:, 0])
            else:
                wload_hi(0, w_cur[0])
            for ti, (to, tw) in enumerate(tiles):
                # some xh tiles ride the ACT queue: a third startup channel
                # (ACT has no compute until the first silu lands, well
                # after these triggers retire)
                xq = nc.scalar if ti in XH_SCALAR else nc.sync
                xq.dma_start(xh_sb[:, :, :, to:to + tw],
                             xh[:, :, :, to:to + tw])
                nc.gpsimd.dma_start(xl_sb[:, :, :, to:to + tw],
                                    xl[:, :, :, to:to + tw])
                if ti == 0:
                    if W0SPLIT:
                        nc.sync.dma_start(w_cur[0][0][:, 1], whi[0][:, 1])
                    wload_lo(0, w_cur[0])
            # h>=1 weights trail the full x stream on both queues: each xh
            # tile is consumed within ~1-2us, while w[h] only gates the
            # next 11us-long h-sweep
            for h in range(1, npre):
                wload_hi(h, w_cur[h])
                wload_lo(h, w_cur[h])

            psa = pha.enter_context(tc.tile_pool(name="psA", bufs=2, space="PSUM"))
            stg = pha.enter_context(tc.tile_pool(name="stg", bufs=3))
            # deep staging: s_lo DMA-out rides the busy SWDGE queue, so the
            # writes may lag the compute by several (h,t) groups
            slop = pha.enter_context(tc.tile_pool(name="slo", bufs=12))

            def emit_ht(h, t, wt):
                nonlocal anchor
                whi_t, wlo_t = wt
                to, tw = tiles[t]
                hd, blk = divmod(h, 2)
                p1 = psa.tile([128, TT], FP32, tag="p1", name=f"p1_{h}_{t}")
                p3 = psa.tile([128, TT], FP32, tag="p3", name=f"p3_{h}_{t}")
                for pp, wh_, wl_ in ((p1, whi_t[:, 0], wlo_t[:, 0]),
                                     (p3, whi_t[:, 1], wlo_t[:, 1])):
                    n = 0
                    for xs_, ws_ in ((xh_sb, wh_), (xl_sb, wh_), (xh_sb, wl_)):
                        for kd in range(NKC):
                            anchor = nc.tensor.matmul(
                                pp[:, :tw], ws_[:, kd],
                                xs_[:, kd, :, to:to + tw],
                                start=(n == 0), stop=(n == 3 * NKC - 1),
                                perf_mode=DR)
                            n += 1
                sa = stg.tile([128, TT], FP32, tag="sa", name=f"sa{h}_{t}")
                nc.scalar.activation(sa[:, :tw], p1[:, :tw], AF.Silu,
                                     scale=1.0 / SA)
                t1 = stg.tile([128, TT], FP32, tag="t1", name=f"t1_{h}_{t}")
                acc = stg.tile([128, 1], FP32, tag="acc", name=f"acc{h}_{t}")
                nc.vector.affine_mul_reduce(t1[:, :tw], acc[:], p3[:, :tw],
                                            sa[:, :tw], SH / SA, 0.0)
                hi_sl = s_hi[:, hd, blk, to:to + tw]
                nc.scalar.activation(hi_sl, t1[:, :tw], AF.Copy)
                slo = slop.tile([128, TT], FP8, tag="slo", name=f"slo{h}_{t}")
                nc.vector.tensor_tensor(slo[:, :tw], t1[:, :tw], hi_sl,
                                        op=ALU.subtract)
                nc.gpsimd.dma_start(slo_dram[t][:, hd, blk, :tw],
                                    slo[:, :tw])

            # emission: preloaded h-levels h-major, ramped tiles ascending
            for h in range(npre):
                for t in range(len(tiles)):
                    emit_ht(h, t, w_cur[h])
            for h in range(npre, KH):
                wt = walloc(h)
                wload_hi(h, wt)
                wload_lo(h, wt)
                w_cur[h] = wt
                j = h - npre
                if j < len(w2_loads):
                    dst, src = w2_loads[j]
                    w2dma = nc.gpsimd.dma_start(dst[:], src)
                    add_dep_helper(w2dma.ins, anchor.ins,
                                   reason="delay w2 prefetch")
                if h == 6:
                    emit_g()
                for t in range(len(tiles)):
                    emit_ht(h, t, w_cur[h])
            # two stragglers (KH - npre = 18 slots for 16 w2 loads) — none

        # ---- phase B: y = 3-term(s @ w2.T) * g ----
        with ExitStack() as phb:
            sin = phb.enter_context(tc.tile_pool(name="sin", bufs=2))
            yp = phb.enter_context(tc.tile_pool(name="yst", bufs=4))
            # big tiles in the middle; the smallest tile LAST so the final
            # y writeback (which trails the last matmul) is the shortest
            b_order = sorted(range(len(tiles)),
                             key=lambda i: (-tiles[i][1], i))
            b_order = b_order[:-1] + [b_order[-1]]
            sm = min(range(len(tiles)), key=lambda i: tiles[i][1])
            b_order = [i for i in b_order if i != sm] + [sm]
            # the first tile's s_lo reload rides SYNC (idle through phase A,
            # and the wait on that tile's writes resolves ~10us before the
            # A/B boundary); later tiles go per-tile on the ACT queue, which
            # frees up right at the boundary
            for bi, t in enumerate(b_order):
                to, tw = tiles[t]
                sl_t = sin.tile([128, NKH, 2, TT], FP8, tag="sl",
                                name=f"sl_sb{t}")
                q = nc.sync if bi == 0 else nc.scalar
                q.dma_start(sl_t[:, :, :, :tw], slo_dram[t][:, :, :, :tw])
                for c in range(KC8):
                    py = psb.tile([128, TT], FP32, tag="py", name=f"py{t}_{c}")
                    n = 0
                    # the s_lo group goes LAST so the chain can start before
                    # the reload DMA of this tile's s_lo has landed
                    for s_, w_ in ((s_hi, w2h_sb[c]), (s_hi, w2l_sb[c]),
                                   (None, w2h_sb[c])):
                        for hd in range(NKH):
                            mv = (sl_t[:, hd, :, :tw] if s_ is None
                                  else s_[:, hd, :, to:to + tw])
                            nc.tensor.matmul(py[:, :tw], w_[:, hd], mv,
                                             start=(n == 0),
                                             stop=(n == 3 * NKH - 1),
                                             perf_mode=DR)
                            n += 1
                    yb = yp.tile([128, TT], FP32, tag="y", name=f"yb{t}_{c}")
                    nc.vector.tensor_tensor(yb[:, :tw], py[:, :tw], g_sb[t][:],
                                            op=ALU.mult)
                    # alternate writeback queues: halves the per-queue y
                    # rate so the final transfer doesn't trail the compute
                    yq = nc.sync if c % 2 == 0 else nc.scalar
                    yq.dma_start(yt[c, :, to:to + tw], yb[:, :tw])

    nc.compile()
    return nc


def _prep_weights(gw, w1, w2, w3):
    """Quantize + arrange all per-expert weight tensors (host, cached)."""
    wmaps = []
    for e in range(E):
        m = {}
        his, los = {}, {}
        for nm, w in (("w1", w1[e]), ("w3", w3[e])):
            wt = np.zeros((HP, C), np.float32)
            wt[:H0] = w
            hi, lo = _split8(wt * SW)
            # [HP, C] -> [KH, 128m, NKC, 2, 128p] -> [KH, 128p, NKC, 2, 128m]
            his[nm] = hi.reshape(KH, 128, NKC, 2, 128).transpose(0, 4, 2, 3, 1)
            los[nm] = lo.reshape(KH, 128, NKC, 2, 128).transpose(0, 4, 2, 3, 1)
        m["whi"] = np.ascontiguousarray(
            np.stack([his["w1"], his["w3"]], axis=2))
        m["wlo"] = np.ascontiguousarray(
            np.stack([los["w1"], los["w3"]], axis=2))
        wt = np.zeros((C, HP), np.float32)
        wt[:, :H0] = w2[e]
        hi, lo = _split8(wt * SW)
        # [C, HP] -> [KC8, 128m, NKH, 2, 128p] -> [KC8, 128p, NKH, 2, 128m]
        for part, arr in (("h", hi), ("l", lo)):
            a = arr.reshape(KC8, 128, NKH, 2, 128).transpose(0, 4, 2, 3, 1)
            m["w2" + part] = np.ascontiguousarray(a)
        wmaps.append(m)
    return wmaps


def kernel(x, gate_w, w1, w2, w3, top_k):
    global LAST_RESULTS
    x = np.asarray(x, dtype=np.float32)
    gw = np.asarray(gate_w, dtype=np.float32)
    w1 = np.asarray(w1, dtype=np.float32)
    w2 = np.asarray(w2, dtype=np.float32)
    w3 = np.asarray(w3, dtype=np.float32)
    assert int(np.asarray(top_k)) == 2
    Bb, T, Cc = x.shape
    N = Bb * T
    assert Cc == C and w1.shape == (E, H0, C)

    xf = np.ascontiguousarray(x.reshape(N, C))
    # Router on host (exact fp32): top-2 selection + softmax combine weights.
    logits = xf @ gw.T
    order = np.argsort(-logits, axis=1, kind="stable")[:, :2]
    vals = np.take_along_axis(logits, order, axis=1)
    sw = np.exp(vals - vals.max(axis=1, keepdims=True))
    sw /= sw.sum(axis=1, keepdims=True)
    tok, gtok = [], []
    for e in range(E):
        sel = order == e
        idx = np.nonzero(sel.any(axis=1))[0]
        tok.append(idx)
        gtok.append(sw[sel].astype(np.float32))

    key = (w1.shape, float(w1[0, 0, :8].sum()), float(w2[-1, -1, :8].sum()),
           float(w3[0, -1, :8].sum()))
    wm = _WCACHE.get(key)
    if wm is None:
        wm = _prep_weights(gw, w1, w2, w3)
        _WCACHE.clear()
        _WCACHE[key] = wm

    # quantize x once (full token set), dispatch indexes the fp8 arrays
    xs = xf * SX
    xh_full, xl_full = _split8(xs)

    out = np.zeros((N, C), np.float32)
    nchunk = (max(t.size for t in tok) + CAP_MAX - 1) // CAP_MAX
    for ci in range(nchunk):
        tokc, gc = [], []
        for e in range(E):
            lo_ = (ci * tok[e].size) // nchunk
            hi_ = ((ci + 1) * tok[e].size) // nchunk
            tokc.append(tok[e][lo_:hi_])
            gc.append(gtok[e][lo_:hi_])
        cap = max(TT, ((max(t.size for t in tokc) + 127) // 128) * 128)
        if cap not in _CACHE:
            _CACHE[cap] = _build(cap)
        nc = _CACHE[cap]
        in_maps = []
        for e in range(E):
            idx = tokc[e]
            n = idx.size
            im = dict(wm[e])
            for nm, full in (("xh", xh_full), ("xl", xl_full)):
                xe = np.zeros((cap, C), E4NP)
                xe[:n] = full[idx]
                # [cap, C] -> [128p, NKC, 2, cap] with c = kd*256+i*128+p
                a = xe.T.reshape(NKC, 2, 128, cap).transpose(2, 0, 1, 3)
                im[nm] = np.ascontiguousarray(a)
            g = np.zeros(cap, np.float32)
            g[:n] = gc[e] / SB
            im["gsc"] = g
            in_maps.append(im)

        res = run_bass_kernel_spmd(nc, in_maps, core_ids=list(range(E)))
        LAST_RESULTS = res

        for e in range(E):
            idx = tokc[e]
            n = idx.size
            ye = res.results[e]["yt"].reshape(C, cap).T
            out[idx] += ye[:n]
    return out.reshape(Bb, T, C)


# revision 57
# speedup vs baseline: 1.3802x; 1.0163x over previous
"""Expert-parallel MoE layer for Trainium2 (Bass/Tile, 8 NeuronCores).

Strategy (hardcoded for B=4, T=2048, C=1024, E=8, H=2728, top_k=2):
  - Expert-parallel: core e owns expert e's weights (w1/w2/w3[e]).
  - Host computes the router (top-2 ids AND the softmax combine weights in
    exact fp32) and performs the all-to-all token dispatch/combine as the
    shard/unshard step. The per-token gate weight rides in as a small fp32
    vector, pre-scaled by the fp8 scale factors.
  - Each core computes the full expert FFN y = (silu(x@w1.T) * (x@w3.T))
    @ w2.T * g for its tokens, entirely in fp8-e4m3 DoubleRow matmuls
    (K=256 per instruction, 0.5 cycles/column — 2x the fp32r MAC rate).

Precision: every GEMM uses a 3-term hi/lo split, all at one shared scale so
the three products accumulate in a single PSUM chain:
    A@W ~= Ah@Wh + Al@Wh + Ah@Wl      (drops only the Al@Wl term, ~7e-4)
with Ah = e4m3(A*S), Al = e4m3(A*S - Ah). End-to-end rel err ~2e-3 vs the
2e-2 gate. x/w splits happen on host; the phase-A output s is split on
device (ACT copy for the hi part, DVE subtract for the residual).

Layouts are pre-arranged on host so every matmul operand is a direct SBUF
slice: stationary tiles [128, 2, 128] (DoubleRow K-pair x M), moving tiles
[128, 2, tw]. s_hi stays resident in SBUF; s_lo spills to DRAM and streams
back during phase B (bandwidth is far under the PE time either way).
"""

import os
import sys
from contextlib import ExitStack

import numpy as np
import ml_dtypes

for _p in ("/opt/trn_rl_repo", "/root/.axon_site/_ro/trn_rl_repo"):
    if os.path.isdir(_p) and _p not in sys.path:
        sys.path.insert(0, _p)

import concourse.mybir as mybir
import concourse.tile as tile
from concourse.tile_rust import add_dep_helper
from concourse import bacc
from concourse.bass_utils import run_bass_kernel_spmd

FP32 = mybir.dt.float32
FP8 = mybir.dt.float8e4
ALU = mybir.AluOpType
AF = mybir.ActivationFunctionType
DR = mybir.MatmulPerfMode.DoubleRow
E4NP = ml_dtypes.float8_e4m3

E = 8            # experts == cores
C = 1024         # model dim
H0 = 2728        # ffn hidden dim
NKC = C // 256   # 4 DoubleRow contraction tiles over C
KH = 22          # 128-row h tiles (padded H)
NKH = KH // 2    # 11 DoubleRow contraction tiles over padded H
HP = KH * 128    # 2816
KC8 = C // 128   # 8 output c tiles
TT = 512         # max token tile (fp32 PSUM bank = 512 floats)
CAP_MAX = 2304   # per-launch token cap (SBUF budget); split into runs beyond

# fp8 scale factors. All hi/lo parts share their tensor's scale so the three
# split products accumulate in one PSUM chain.
SX = 16.0        # x*16: |x|<5.1 -> <82, x_lo ~0.4 (normal range)
SW = 1024.0      # w*1024: |w|<0.11 -> <113
SH = 8.0         # s*8: |s|<12 -> <96 (clip-safe), s_lo ~0.07
SA = SX * SW     # phase-A psum scale
SB = SH * SW     # phase-B psum scale

_CACHE = {}
_WCACHE = {}
LAST_RESULTS = None

# startup-schedule knobs (fixed by a TimelineSim sweep)
XH_SCALAR = (1, 3)      # xh tile indices that ride the ACT queue
W0SPLIT = True          # split whi[0] into w1/w3 halves around xh0
RAMP = (128, 256, 256)  # leading token-tile widths


def _token_tiles(cap):
    # sub-512 tiles go FIRST (ascending): the first matmuls' DMA
    # dependencies are smaller, so the PE starts (and ramps) earlier. A
    # 128-wide leader is fine at fp8-DR (no narrow-tile rate penalty).
    ramp = list(RAMP)
    while sum(ramp) > max(0, cap - 256) and len(ramp) > 1:
        ramp.pop()
    widths = list(ramp)
    left = cap - sum(ramp)
    if left % TT:
        widths.append(left % TT)
    widths += [TT] * (left // TT)
    widths.sort()
    tiles = []
    off = 0
    for w in widths:
        tiles.append((off, w))
        off += w
    return tiles


def _split8(a):
    """a is pre-scaled fp32; return (hi, lo) e4m3 arrays at the same scale."""
    hi = np.clip(a, -240.0, 240.0).astype(E4NP)
    lo = (a - hi.astype(np.float32)).astype(E4NP)
    return hi, lo


def _build(cap):
    """Build + compile the SPMD program for `cap` tokens per core."""
    assert cap % 128 == 0
    tiles = _token_tiles(cap)
    last = len(tiles) - 1
    nc = bacc.Bacc("TRN2", target_bir_lowering=False, debug=False, num_devices=E)

    # x packed tile-major: per (partition, token-tile) the [NKC, 2, tw]
    # block is contiguous, so every tile's DMA moves >=2KB chunks (chunks
    # under 512B pay a 2x DMA-bus penalty in HW)
    xh = nc.dram_tensor("xh", [128, NKC * 2 * cap], FP8, kind="ExternalInput").ap()
    xl = nc.dram_tensor("xl", [128, NKC * 2 * cap], FP8, kind="ExternalInput").ap()
    # w1h+w3h (resp. w1l+w3l) fused per h-tile: one DMA instead of two
    # (fixed cost per DMA dominates these small transfers)
    whi = nc.dram_tensor("whi", [KH, 128, 2, NKC, 2, 128], FP8, kind="ExternalInput").ap()
    wlo = nc.dram_tensor("wlo", [KH, 128, 2, NKC, 2, 128], FP8, kind="ExternalInput").ap()
    w2h = nc.dram_tensor("w2h", [KC8, 128, NKH, 2, 128], FP8, kind="ExternalInput").ap()
    w2l = nc.dram_tensor("w2l", [KC8, 128, NKH, 2, 128], FP8, kind="ExternalInput").ap()
    gsc = nc.dram_tensor("gsc", [cap], FP32, kind="ExternalInput").ap()
    yt = nc.dram_tensor("yt", [KC8, 128, cap], FP32, kind="ExternalOutput").ap()

    with tile.TileContext(nc) as tc, ExitStack() as top:
        dramp = top.enter_context(tc.tile_pool(name="dram", bufs=1, space="DRAM"))
        # one scratch tensor per token tile so the phase-B reload of tile t
        # only depends on tile t's writes, not the whole phase A
        ntile = len(tiles)
        slo_dram = [dramp.tile([128, NKH, 2, TT], FP8, tag=f"slo{t}",
                               name=f"slo_dram{t}")
                    for t in range(ntile)]

        shp = top.enter_context(tc.tile_pool(name="sres", bufs=1))
        s_hi = shp.tile([128, NKH, 2, cap], FP8)

        # w2 resident for the whole kernel; loads interleaved into phase A's
        # h-loop so they hide behind compute without starving startup DMA
        w2p = top.enter_context(tc.tile_pool(name="w2res", bufs=1))
        w2h_sb = [w2p.tile([128, NKH, 2, 128], FP8, tag=f"w2h{c}",
                           name=f"w2h_sb{c}") for c in range(KC8)]
        w2l_sb = [w2p.tile([128, NKH, 2, 128], FP8, tag=f"w2l{c}",
                           name=f"w2l_sb{c}") for c in range(KC8)]
        w2_loads = [(w2h_sb[c], w2h[c]) for c in range(KC8)] + \
                   [(w2l_sb[c], w2l[c]) for c in range(KC8)]

        gbc = top.enter_context(tc.tile_pool(name="gbc", bufs=1))
        g_sb = []

        def emit_g():
            # gate-weight rows: tiny loads + partition broadcasts on the
            # SWDGE queue, emitted mid-phase-A where that queue has slack —
            # NOT at the phase boundary, where they'd sit behind the s_lo
            # write backlog and stall the first y-multiplies
            for t, (to, tw) in enumerate(tiles):
                grow = gbc.tile([1, TT], FP32, tag="grow", name=f"grow{t}",
                                bufs=2)
                nc.gpsimd.dma_start(grow[0:1, :tw], gsc[to:to + tw])
                gt = gbc.tile([128, tw], FP32, tag=f"g{t}", name=f"g_sb{t}")
                nc.gpsimd.partition_broadcast(gt[:], grow[0:1, :tw])
                g_sb.append(gt)

        # phase B's PSUM pool is allocated up front so it lands in banks
        # disjoint from phase A's — otherwise B's first chain waits ~1us
        # for A's tail to release a recycled bank
        psb = top.enter_context(tc.tile_pool(name="psB", bufs=3, space="PSUM"))
        anchor = None
        with ExitStack() as pha:
            xp = pha.enter_context(tc.tile_pool(name="xres", bufs=1))
            xh_sb = [xp.tile([128, NKC, 2, tw], FP8, tag=f"xh{t}",
                             name=f"xh_sb{t}") for t, (to, tw) in enumerate(tiles)]
            xl_sb = [xp.tile([128, NKC, 2, tw], FP8, tag=f"xl{t}",
                             name=f"xl_sb{t}") for t, (to, tw) in enumerate(tiles)]
            wst = pha.enter_context(tc.tile_pool(name="wst", bufs=4))

            def walloc(h):
                return (
                    wst.tile([128, 2, NKC, 2, 128], FP8, tag="whi", name=f"whi_{h}"),
                    wst.tile([128, 2, NKC, 2, 128], FP8, tag="wlo", name=f"wlo_{h}"),
                )

            # startup streams split across the two free queues in exact
            # consumption order (per-queue DMA processing is FIFO): SYNC
            # carries the hi parts (consumed first in every chain) + xh;
            # the gpsimd/SWDGE queue carries xl + the fused lo parts. The
            # ACT queue must stay empty here: each DMA on it would occupy
            # the ACT sequencer ~1.3us and push the silu/quantize chain
            # (and with it PSUM recycling) out by that much. x rides ahead
            # of the h>=1 weights: each xh tile is consumed ~1us after the
            # previous, while w[h] only gates the next 11us-long h-sweep.
            def wload_hi(h, wt):
                nc.sync.dma_start(wt[0][:], whi[h])

            def wload_lo(h, wt):
                nc.gpsimd.dma_start(wt[1][:], wlo[h])

            # only h0/h1 preload: the startup DMA wall is the global
            # DMA-engine bandwidth, so deferring h2+ weights (needed only
            # ~22us in) out of the startup window shrinks the stall
            npre = min(2, KH)
            w_cur = {h: walloc(h) for h in range(npre)}
            # h0's hi weights optionally split in two: the first chain only
            # needs the w1 half, so it rides ahead of xh0, w3 follows
            if W0SPLIT:
                nc.sync.dma_start(w_cur[0][0][:, 0], whi[0][:, 0])
            else:
                wload_hi(0, w_cur[0])
            for ti, (to, tw) in enumerate(tiles):
                # some xh tiles ride the ACT queue: a third startup channel
                # (ACT has no compute until the first silu lands, well
                # after these triggers retire)
                xq = nc.scalar if ti in XH_SCALAR else nc.sync
                xq.dma_start(
                    xh_sb[ti][:],
                    xh[:, 8 * to:8 * (to + tw)].rearrange(
                        "p (k i t) -> p k i t", k=NKC, i=2))
                nc.gpsimd.dma_start(
                    xl_sb[ti][:],
                    xl[:, 8 * to:8 * (to + tw)].rearrange(
                        "p (k i t) -> p k i t", k=NKC, i=2))
                if ti == 0:
                    if W0SPLIT:
                        nc.sync.dma_start(w_cur[0][0][:, 1], whi[0][:, 1])
                    wload_lo(0, w_cur[0])
                    wload_hi(1, w_cur[1])
                    wload_lo(1, w_cur[1])

            psa = pha.enter_context(tc.tile_pool(name="psA", bufs=2, space="PSUM"))
            stg = pha.enter_context(tc.tile_pool(name="stg", bufs=3))
            # deep staging: s_lo DMA-out rides the busy SWDGE queue, so the
            # writes may lag the compute by several (h,t) groups
            slop = pha.enter_context(tc.tile_pool(name="slo", bufs=12))

            def emit_ht(h, t, wt):
                nonlocal anchor
                whi_t, wlo_t = wt
                to, tw = tiles[t]
                hd, blk = divmod(h, 2)
                p1 = psa.tile([128, TT], FP32, tag="p1", name=f"p1_{h}_{t}")
                p3 = psa.tile([128, TT], FP32, tag="p3", name=f"p3_{h}_{t}")
                xh_t, xl_t = xh_sb[t], xl_sb[t]
                for pp, wh_, wl_ in ((p1, whi_t[:, 0], wlo_t[:, 0]),
                                     (p3, whi_t[:, 1], wlo_t[:, 1])):
                    n = 0
                    for xs_, ws_ in ((xh_t, wh_), (xl_t, wh_), (xh_t, wl_)):
                        for kd in range(NKC):
                            anchor = nc.tensor.matmul(
                                pp[:, :tw], ws_[:, kd],
                                xs_[:, kd],
                                start=(n == 0), stop=(n == 3 * NKC - 1),
                                perf_mode=DR)
                            n += 1
                sa = stg.tile([128, TT], FP32, tag="sa", name=f"sa{h}_{t}")
                nc.scalar.activation(sa[:, :tw], p1[:, :tw], AF.Silu,
                                     scale=1.0 / SA)
                t1 = stg.tile([128, TT], FP32, tag="t1", name=f"t1_{h}_{t}")
                acc = stg.tile([128, 1], FP32, tag="acc", name=f"acc{h}_{t}")
                nc.vector.affine_mul_reduce(t1[:, :tw], acc[:], p3[:, :tw],
                                            sa[:, :tw], SH / SA, 0.0)
                hi_sl = s_hi[:, hd, blk, to:to + tw]
                nc.scalar.activation(hi_sl, t1[:, :tw], AF.Copy)
                slo = slop.tile([128, TT], FP8, tag="slo", name=f"slo{h}_{t}")
                nc.vector.tensor_tensor(slo[:, :tw], t1[:, :tw], hi_sl,
                                        op=ALU.subtract)
                nc.gpsimd.dma_start(slo_dram[t][:, hd, blk, :tw],
                                    slo[:, :tw])

            # emission: h0/h1 interleaved token-major — each arriving x tile
            # feeds 2x the PE work, halving the startup feed-rate demand on
            # the global DMA engines
            for t in range(len(tiles)):
                for h in range(npre):
                    emit_ht(h, t, w_cur[h])
            for h in range(npre, KH):
                wt = walloc(h)
                wload_hi(h, wt)
                wload_lo(h, wt)
                w_cur[h] = wt
                j = h - npre
                if j < len(w2_loads):
                    dst, src = w2_loads[j]
                    w2dma = nc.gpsimd.dma_start(dst[:], src)
                    add_dep_helper(w2dma.ins, anchor.ins,
                                   reason="delay w2 prefetch")
                if h == 6:
                    emit_g()
                for t in range(len(tiles)):
                    emit_ht(h, t, w_cur[h])
            # two stragglers (KH - npre = 18 slots for 16 w2 loads) — none

        # ---- phase B: y = 3-term(s @ w2.T) * g ----
        with ExitStack() as phb:
            sin = phb.enter_context(tc.tile_pool(name="sin", bufs=2))
            yp = phb.enter_context(tc.tile_pool(name="yst", bufs=4))
            # big tiles in the middle; the smallest tile LAST so the final
            # y writeback (which trails the last matmul) is the shortest
            b_order = sorted(range(len(tiles)),
                             key=lambda i: (-tiles[i][1], i))
            b_order = b_order[:-1] + [b_order[-1]]
            sm = min(range(len(tiles)), key=lambda i: tiles[i][1])
            b_order = [i for i in b_order if i != sm] + [sm]
            # the first tile's s_lo reload rides SYNC (idle through phase A,
            # and the wait on that tile's writes resolves ~10us before the
            # A/B boundary); later tiles go per-tile on the ACT queue, which
            # frees up right at the boundary
            for bi, t in enumerate(b_order):
                to, tw = tiles[t]
                sl_t = sin.tile([128, NKH, 2, TT], FP8, tag="sl",
                                name=f"sl_sb{t}")
                q = nc.sync if bi == 0 else nc.scalar
                q.dma_start(sl_t[:, :, :, :tw], slo_dram[t][:, :, :, :tw])
                for c in range(KC8):
                    py = psb.tile([128, TT], FP32, tag="py", name=f"py{t}_{c}")
                    n = 0
                    # the s_lo group goes LAST so the chain can start before
                    # the reload DMA of this tile's s_lo has landed
                    for s_, w_ in ((s_hi, w2h_sb[c]), (s_hi, w2l_sb[c]),
                                   (None, w2h_sb[c])):
                        for hd in range(NKH):
                            mv = (sl_t[:, hd, :, :tw] if s_ is None
                                  else s_[:, hd, :, to:to + tw])
                            nc.tensor.matmul(py[:, :tw], w_[:, hd], mv,
                                             start=(n == 0),
                                             stop=(n == 3 * NKH - 1),
                                             perf_mode=DR)
                            n += 1
                    yb = yp.tile([128, TT], FP32, tag="y", name=f"yb{t}_{c}")
                    nc.vector.tensor_tensor(yb[:, :tw], py[:, :tw], g_sb[t][:],
                                            op=ALU.mult)
                    # alternate writeback queues: halves the per-queue y
                    # rate so the final transfer doesn't trail the compute
                    yq = nc.sync if c % 2 == 0 else nc.scalar
                    yq.dma_start(yt[c, :, to:to + tw], yb[:, :tw])

    nc.compile()
    return nc


def _prep_weights(gw, w1, w2, w3):
    """Quantize + arrange all per-expert weight tensors (host, cached)."""
    wmaps = []
    for e in range(E):
        m = {}
        his, los = {}, {}
        for nm, w in (("w1", w1[e]), ("w3", w3[e])):
            wt = np.zeros((HP, C), np.float32)
            wt[:H0] = w
            hi, lo = _split8(wt * SW)
            # [HP, C] -> [KH, 128m, NKC, 2, 128p] -> [KH, 128p, NKC, 2, 128m]
            his[nm] = hi.reshape(KH, 128, NKC, 2, 128).transpose(0, 4, 2, 3, 1)
            los[nm] = lo.reshape(KH, 128, NKC, 2, 128).transpose(0, 4, 2, 3, 1)
        m["whi"] = np.ascontiguousarray(
            np.stack([his["w1"], his["w3"]], axis=2))
        m["wlo"] = np.ascontiguousarray(
            np.stack([los["w1"], los["w3"]], axis=2))
        wt = np.zeros((C, HP), np.float32)
        wt[:, :H0] = w2[e]
        hi, lo = _split8(wt * SW)
        # [C, HP] -> [KC8, 128m, NKH, 2, 128p] -> [KC8, 128p, NKH, 2, 128m]
        for part, arr in (("h", hi), ("l", lo)):
            a = arr.reshape(KC8, 128, NKH, 2, 128).transpose(0, 4, 2, 3, 1)
            m["w2" + part] = np.ascontiguousarray(a)
        wmaps.append(m)
    return wmaps


def kernel(x, gate_w, w1, w2, w3, top_k):
    global LAST_RESULTS
    x = np.asarray(x, dtype=np.float32)
    gw = np.asarray(gate_w, dtype=np.float32)
    w1 = np.asarray(w1, dtype=np.float32)
    w2 = np.asarray(w2, dtype=np.float32)
    w3 = np.asarray(w3, dtype=np.float32)
    assert int(np.asarray(top_k)) == 2
    Bb, T, Cc = x.shape
    N = Bb * T
    assert Cc == C and w1.shape == (E, H0, C)

    xf = np.ascontiguousarray(x.reshape(N, C))
    # Router on host (exact fp32): top-2 selection + softmax combine weights.
    logits = xf @ gw.T
    order = np.argsort(-logits, axis=1, kind="stable")[:, :2]
    vals = np.take_along_axis(logits, order, axis=1)
    sw = np.exp(vals - vals.max(axis=1, keepdims=True))
    sw /= sw.sum(axis=1, keepdims=True)
    tok, gtok = [], []
    for e in range(E):
        sel = order == e
        idx = np.nonzero(sel.any(axis=1))[0]
        tok.append(idx)
        gtok.append(sw[sel].astype(np.float32))

    key = (w1.shape, float(w1[0, 0, :8].sum()), float(w2[-1, -1, :8].sum()),
           float(w3[0, -1, :8].sum()))
    wm = _WCACHE.get(key)
    if wm is None:
        wm = _prep_weights(gw, w1, w2, w3)
        _WCACHE.clear()
        _WCACHE[key] = wm

    # quantize x once (full token set), dispatch indexes the fp8 arrays
    xs = xf * SX
    xh_full, xl_full = _split8(xs)

    out = np.zeros((N, C), np.float32)
    nchunk = (max(t.size for t in tok) + CAP_MAX - 1) // CAP_MAX
    for ci in range(nchunk):
        tokc, gc = [], []
        for e in range(E):
            lo_ = (ci * tok[e].size) // nchunk
            hi_ = ((ci + 1) * tok[e].size) // nchunk
            tokc.append(tok[e][lo_:hi_])
            gc.append(gtok[e][lo_:hi_])
        cap = max(TT, ((max(t.size for t in tokc) + 127) // 128) * 128)
        if cap not in _CACHE:
            _CACHE[cap] = _build(cap)
        nc = _CACHE[cap]
        in_maps = []
        for e in range(E):
            idx = tokc[e]
            n = idx.size
            im = dict(wm[e])
            tls = _token_tiles(cap)
            for nm, full in (("xh", xh_full), ("xl", xl_full)):
                xe = np.zeros((cap, C), E4NP)
                xe[:n] = full[idx]
                # [cap, C] -> [128p, NKC, 2, cap] with c = kd*256+i*128+p,
                # then packed tile-major: per partition the [NKC, 2, tw]
                # block of each token tile is contiguous
                a = xe.T.reshape(NKC, 2, 128, cap).transpose(2, 0, 1, 3)
                im[nm] = np.concatenate(
                    [np.ascontiguousarray(a[:, :, :, to:to + tw]).reshape(128, -1)
                     for to, tw in tls], axis=1)
            g = np.zeros(cap, np.float32)
            g[:n] = gc[e] / SB
            im["gsc"] = g
            in_maps.append(im)

        res = run_bass_kernel_spmd(nc, in_maps, core_ids=list(range(E)))
        LAST_RESULTS = res

        for e in range(E):
            idx = tokc[e]
            n = idx.size
            ye = res.results[e]["yt"].reshape(C, cap).T
            out[idx] += ye[:n]
    return out.reshape(Bb, T, C)
